# revision 48
# baseline (speedup 1.0000x reference)
import sys

sys.path.insert(0, "/opt/trn_rl_repo")

from contextlib import ExitStack

import numpy as np

import concourse.bass as bass
import concourse.tile as tile
from concourse import bacc
from concourse import mybir

B, T, C = 4, 2048, 1024
NH, D = 16, 64
NCORES = 8
BP = 2            # batch-pair shards (2 batches each)
HG = 4            # head-group shards (4 heads each)
B_LOC = 2         # batches per core
HPC = 4           # heads per core
HP = 2            # head-pairs per core
F = HPC * D       # per-core feature slice (256)
P = 128
TCH = 512         # token chunk (qkv) == q chunk (attention)
KT = 128          # k tile
f32 = mybir.dt.float32
f16 = mybir.dt.float16
AF = mybir.ActivationFunctionType

# quadrant-local rope layout: each 32-partition quadrant holds 16 x1 slots
# then 16 x2 slots; the rotation becomes a +/-16 shuffle within the quadrant.
ROPE_PERM = list(range(0, 16)) + list(range(32, 48)) + \
    list(range(16, 32)) + list(range(48, 64))
SHUF_MASK = [(i + 16) % 32 for i in range(32)]

MM_LABELS = []  # emission-order matmul labels (profiling aid)

# PE-time estimates (ns) for pacing filler work into attention slots
PEC = 1e9 / 2.4e9          # pe cycle at full speed
MM_OVH = 35.0
QK_NS = 8 * (512 * PEC + MM_OVH)
V_NS = 8 * (256 * PEC + MM_OVH)
OPH_NS = 2 * (512 * PEC + MM_OVH)
PACE = 1.5                # fillers pace at PE rate (PE-bound), not ACT rate


def build_nc():
    """One-core SPMD program: 2 batches x 4 heads (2 head-pairs).

    f16 compute throughout (fp8 cannot meet the accuracy gate on randn
    inputs: quantization error does not average out of attention). The
    attention loop is software-pipelined with S one k-tile ahead; the
    causal mask is folded into the score psum via an identity@bias
    matmul so no vector-engine mask op is needed; qkv + out-proj
    matmuls are paced into the slots by a PE-time credit model.
    """
    t = T
    bt = B_LOC * t            # 4096 tokens
    n_cc = C // P             # 8 contraction chunks

    nc = bacc.Bacc(None, target_bir_lowering=False)
    xT = nc.declare_dram_parameter("xT", [C, bt], f16, isOutput=False)
    wq = nc.declare_dram_parameter("wq", [C, F], f16, isOutput=False)
    wk = nc.declare_dram_parameter("wk", [C, F], f16, isOutput=False)
    wv = nc.declare_dram_parameter("wv", [C, F], f16, isOutput=False)
    wo = nc.declare_dram_parameter("wo", [F, C], f16, isOutput=False)
    cos2 = nc.declare_dram_parameter("cos2", [P, t], f16, isOutput=False)
    sinS = nc.declare_dram_parameter("sinS", [P, t], f16, isOutput=False)
    mask0 = nc.declare_dram_parameter("mask0", [P, 2 * P], f16, isOutput=False)
    y = nc.declare_dram_parameter("y", [bt, C], f16, isOutput=True)

    xT_t = xT.rearrange("(o p) n -> p o n", p=P)     # [128, 8, 4096]
    wq_r = wq.rearrange("(o p) f -> p o f", p=P)     # [128, 8, 256]
    wk_r = wk.rearrange("(o p) f -> p o f", p=P)
    wv_r = wv.rearrange("(o p) f -> p o f", p=P)
    wo_r = wo.rearrange("(hp p) c -> p hp c", p=P)   # [128, 2, 1024]

    MM_LABELS.clear()

    def MM(label, *a, **kw):
        MM_LABELS.append(label)
        nc.tensor.matmul(*a, **kw)

    with tile.TileContext(nc) as tc, ExitStack() as ctx:
        consts = ctx.enter_context(tc.tile_pool(name="consts", bufs=1))
        xpool = ctx.enter_context(tc.tile_pool(name="xt", bufs=3))
        stage = ctx.enter_context(tc.tile_pool(name="stage", bufs=3))
        ppool = ctx.enter_context(tc.tile_pool(name="pp", bufs=3))
        opool = ctx.enter_context(tc.tile_pool(name="op", bufs=2))
        ypool = ctx.enter_context(tc.tile_pool(name="yst", bufs=3))
        pss = ctx.enter_context(tc.tile_pool(name="pss", bufs=2, space="PSUM"))
        pso = ctx.enter_context(tc.tile_pool(name="pso", bufs=1, space="PSUM"))
        pmix = ctx.enter_context(tc.tile_pool(name="pmix", bufs=2, space="PSUM"))

        # constants; wk + chunk-0 x go out first (k items lead). The first
        # k matmuls accumulate over contraction chunks in order, so loading
        # wk's ft0 columns and xt0's low chunks first lets the PE start
        # ~5us earlier.
        wk_sb = consts.tile([P, n_cc, F], f16)
        nc.sync.dma_start(wk_sb[:, :, 0:P], wk_r[:, :, 0:P])
        xt0 = xpool.tile([P, n_cc, TCH], f16, tag="xt", name="xt0")
        nc.sync.dma_start(xt0[:, 0:4, :], xT_t[:, 0:4, 0:TCH])
        nc.sync.dma_start(xt0[:, 4:8, :], xT_t[:, 4:8, 0:TCH])
        nc.sync.dma_start(wk_sb[:, :, P:F], wk_r[:, :, P:F])
        wq_sb = consts.tile([P, n_cc, F], f16)
        nc.scalar.dma_start(wq_sb, wq_r)
        cos_sb = consts.tile([P, t], f16)
        nc.scalar.dma_start(cos_sb, cos2[:, :])
        sin_sb = consts.tile([P, t], f16)
        nc.scalar.dma_start(sin_sb, sinS[:, :])
        mask_sb = consts.tile([P, 2 * P], f16)
        nc.scalar.dma_start(mask_sb, mask0[:, :])
        wv_sb = consts.tile([P, n_cc, F], f16)
        nc.scalar.dma_start(wv_sb, wv_r)
        wo_sb = consts.tile([P, HP, C], f16)
        nc.scalar.dma_start(wo_sb, wo_r)

        # per-batch persistent tensors
        q_sb = [consts.tile([P, HP, t], f16, name=f"q{b}") for b in range(B_LOC)]
        k_sb = [consts.tile([P, HP, t], f16, name=f"k{b}") for b in range(B_LOC)]
        v1 = [consts.tile([P, t // KT, HPC, D + 1], f16, name=f"v1{b}")
              for b in range(B_LOC)]
        OT = [consts.tile([P, HP, t], f16, name=f"ot{b}") for b in range(B_LOC)]
        for b in range(B_LOC):
            nc.vector.memset(v1[b][:, :, :, D], 1.0)

        # ---- qkv filler items --------------------------------------------
        def rope_store(which, ft, b, c, ps):
            raw = stage.tile([P, TCH], f16, tag="raw")
            nc.vector.tensor_copy(raw, ps)
            rot = stage.tile([P, TCH], f16, tag="rot")
            nc.vector.stream_shuffle(rot, raw, SHUF_MASK)
            dst = (q_sb if which == "q" else k_sb)[b][:, ft, c * TCH : (c + 1) * TCH]
            cs = cos_sb[:, c * TCH : (c + 1) * TCH]
            sn = sin_sb[:, c * TCH : (c + 1) * TCH]
            nc.vector.tensor_mul(out=dst, in0=raw, in1=cs)
            tmp = stage.tile([P, TCH], f16, tag="tmp")
            nc.vector.tensor_mul(out=tmp, in0=rot, in1=sn)
            nc.vector.tensor_add(out=dst, in0=dst, in1=tmp)

        def chunk_items(b, c, split_x=False):
            """qkv for chunk c of batch b. Returns (dma_item, items);
            items are (pe_ns, fn) fillers; the DMA is a prefetch."""
            hold = {}
            items = []
            t0 = b * t + c * TCH
            dma_item = None

            if split_x:
                hold["xt"] = xt0           # preloaded (chunk 0 only)
            else:
                def dma_item():
                    xt = xpool.tile([P, n_cc, TCH], f16, tag="xt")
                    nc.scalar.dma_start(xt, xT_t[:, :, t0 : t0 + TCH])
                    hold["xt"] = xt

            def qk_item(which, ft):
                w_sb = wq_sb if which == "q" else wk_sb
                ps = pmix.tile([P, TCH], f32, tag="px")
                for cc in range(n_cc):
                    MM(
                        f"QKV:{b}.{c}:{which}{ft}",
                        ps, lhsT=w_sb[:, cc, ft * P : (ft + 1) * P],
                        rhs=hold["xt"][:, cc, :], start=(cc == 0),
                        stop=(cc == n_cc - 1),
                    )
                rope_store(which, ft, b, c, ps)

            def v_item(tt):
                ps = pmix.tile([P, TCH], f32, tag="px")
                for cc in range(n_cc):
                    MM(
                        f"V:{b}.{c}:{tt}",
                        ps[:, 0:F],
                        lhsT=hold["xt"][:, cc, tt * P : (tt + 1) * P],
                        rhs=wv_sb[:, cc, :], start=(cc == 0),
                        stop=(cc == n_cc - 1),
                    )
                src = ps[:, 0:F].rearrange("p (h d) -> p h d", d=D)
                nc.vector.tensor_copy(v1[b][:, c * 4 + tt, :, 0:D], src)

            for ft in range(HP):
                items.append((QK_NS, lambda ft=ft: qk_item("k", ft)))
                items.append((QK_NS, lambda ft=ft: qk_item("q", ft)))
            for tt in range(TCH // P):
                items.append((V_NS, lambda tt=tt: v_item(tt)))
            return dma_item, items, qk_item, v_item

        def op_item(b, jj, qt):
            q0 = jj * TCH + qt * P
            r0 = b * t + q0
            ysb = ypool.tile([P, C], f16, tag="ysb")
            for nh in range(2):
                psY = pmix.tile([P, TCH], f32, tag="px")
                for hp in range(HP):
                    MM(
                        f"OP:{b}.{jj}:{qt}",
                        psY, lhsT=OT[b][:, hp, q0 : q0 + P],
                        rhs=wo_sb[:, hp, nh * 512 : (nh + 1) * 512],
                        start=(hp == 0), stop=(hp == HP - 1),
                    )
                nc.vector.tensor_copy(ysb[:, nh * 512 : (nh + 1) * 512], psY)
            nc.gpsimd.dma_start(y[r0 : r0 + P, :], ysb)

        def outproj_items(b, jj):
            units = []
            for qt in range(4):
                holder = {}

                def h0(qt=qt, holder=holder):
                    q0 = jj * TCH + qt * P
                    ysb = ypool.tile([P, C], f16, tag="ysb")
                    holder["ysb"] = ysb
                    psY = pmix.tile([P, TCH], f32, tag="px")
                    for hp in range(HP):
                        MM(f"OP:{b}.{jj}:{qt}", psY,
                           lhsT=OT[b][:, hp, q0 : q0 + P],
                           rhs=wo_sb[:, hp, 0:512],
                           start=(hp == 0), stop=(hp == HP - 1))
                    nc.vector.tensor_copy(ysb[:, 0:512], psY)

                def h1(qt=qt, holder=holder):
                    q0 = jj * TCH + qt * P
                    r0 = b * t + q0
                    ysb = holder["ysb"]
                    psY = pmix.tile([P, TCH], f32, tag="px")
                    for hp in range(HP):
                        MM(f"OP:{b}.{jj}:{qt}", psY,
                           lhsT=OT[b][:, hp, q0 : q0 + P],
                           rhs=wo_sb[:, hp, 512:1024],
                           start=(hp == 0), stop=(hp == HP - 1))
                    nc.vector.tensor_copy(ysb[:, 512:1024], psY)
                    nc.gpsimd.dma_start(y[r0 : r0 + P, :], ysb)

                units.append((OPH_NS, h0))
                units.append((OPH_NS, h1))
            return units

        # ---- attention slot ----------------------------------------------
        # software-pipelined: S one k-tile ahead of the exp stream; the
        # causal bias rides into the diag psum via identity@bias matmuls.
        def attn_slot(b, hp, jj, mid, late, inline_op=False, credit0=0.0):
            n_kt = 4 * jj + 4
            psO = pso.tile([P, 4, 256], f32, tag="po")
            psS_t = [None] * n_kt
            Pp_t = [None] * n_kt

            def S_emit(i):
                lo = max(0, (i - 4 * jj) * KT)
                diag = i >= 4 * jj
                psS = pss.tile([P, 2, TCH], f32, tag="ps")
                for h in range(2):
                    MM(
                        f"S:{b}.{hp}.{jj}:{i}",
                        psS[:, h, lo:],
                        lhsT=k_sb[b][h * D : (h + 1) * D, hp,
                                     i * KT : (i + 1) * KT],
                        rhs=q_sb[b][h * D : (h + 1) * D, hp,
                                    jj * TCH + lo : (jj + 1) * TCH],
                        start=True, stop=not diag,
                    )
                    if diag:
                        # psS[key, tok] += -400 above the causal diagonal:
                        # full-rank constant add (identity @ bias); exp then
                        # underflows to exact f16 zeros - no mask op needed
                        MM(
                            f"SM:{b}.{hp}.{jj}:{i}",
                            psS[:, h, lo : lo + P],
                            lhsT=mask_sb[:, 0:P], rhs=mask_sb[:, P : 2 * P],
                            start=False, stop=True,
                        )
                psS_t[i] = (psS, lo)
                return 2 * ((TCH - lo) * PEC + MM_OVH) + (176 if diag else 0)

            def exp_emit(i):
                psS, lo = psS_t[i]
                Pp = ppool.tile([P, 2, TCH], f16, tag="pp")
                Pp_t[i] = Pp
                nc.scalar.activation(Pp[:, :, lo:], psS[:, :, lo:],
                                     AF.Exp, scale=0.125)
                psS_t[i] = None
                return (2 * (TCH - lo) / 1.2 + 185.0) * PACE

            def PV_emit(i):
                Pp = Pp_t[i]
                lo = max(0, (i - 4 * jj) * KT)
                for qt in range(lo // P, 4):
                    for h in range(2):
                        MM(
                            f"PV:{b}.{hp}.{jj}:{i}",
                            psO[:, qt, h * 65 : h * 65 + 65],
                            lhsT=Pp[:, h, qt * P : (qt + 1) * P],
                            rhs=v1[b][:, i, hp * 2 + h, :],
                            start=(i == 0 and h == 0 and qt % 2 == 0),
                            stop=(i == 4 * jj + qt and h == 1 and qt % 2 == 1),
                        )
                return (4 - lo // P) * 2 * (65 * PEC + MM_OVH)

            def norm_pair(pr):
                rec = stage.tile([P, 2, 2], f32, tag=f"rec{pr}")
                nc.vector.reciprocal(
                    rec, psO[:, 2 * pr : 2 * pr + 2, D : 2 * D + 2 : D + 1])
                Osb = opool.tile([P, 2, 2, D], f16, tag=f"osb{pr}")
                src = psO[:, 2 * pr : 2 * pr + 2, 0 : 2 * (D + 1)].rearrange(
                    "p q (h e) -> p q h e", e=D + 1)[:, :, :, 0:D]
                nc.vector.tensor_mul(
                    out=Osb, in0=src,
                    in1=rec[:, :, :, None].to_broadcast((P, 2, 2, D)),
                )
                for qx in range(2):
                    qt = 2 * pr + qx
                    nc.sync.dma_start_transpose(
                        OT[b][:, hp, jj * TCH + qt * P : jj * TCH + (qt + 1) * P],
                        Osb[:, qx, :, :],
                    )
                    if inline_op:
                        op_item(b, jj, qt)

            credit = credit0
            mq, lq = list(mid), late
            late_gate = 1

            def drain(i):
                nonlocal credit
                while True:
                    if mq and credit >= mq[0][0]:
                        cost, fn = mq.pop(0)
                    elif lq and i >= late_gate and credit >= lq[0][0]:
                        cost, fn = lq.pop(0)
                    else:
                        break
                    fn()
                    credit -= cost
                # deadline pressure: mids must finish this slot — spread the
                # forced drain over the remaining iterations instead of
                # flushing everything after the last PV
                rem = n_kt - i - 1
                while len(mq) > 2 * rem:
                    cost, fn = mq.pop(0)
                    fn()
                    credit -= cost

            S_emit(0)
            for i in range(n_kt):
                if i + 1 < n_kt:
                    credit -= S_emit(i + 1)
                credit += exp_emit(i)
                drain(i)
                credit -= PV_emit(i)
                if i == 4 * jj + 1:
                    norm_pair(0)
            # norm first so the next slot's psO reuse isn't stuck behind
            # the flushed chunk work on the vector engine
            norm_pair(1)
            # chunk (mid) items have a hard deadline at the next slot: flush.
            # out-proj (late) leftovers carry forward to later slots.
            for cost, fn in mq:
                fn()
                credit -= cost
            return max(credit, 0.0)

        # ---- schedule -----------------------------------------------------
        # slots s = b*8 + jj*2 + hp; chunk compute lands one slot before its
        # deadline with its x-load prefetched another slot earlier; out-proj
        # fills later slots (its OT transposes need time to land).
        slot_mid = {s: [] for s in range(16)}
        slot_late = {s: [] for s in range(16)}

        def place_chunk(b, c, s):
            # q/k items at slot s (two slots before the deadline), v items
            # at s+1 (one slot before): the end-of-slot flush then makes
            # every item's data strictly precede its first reader.
            dma, comp, _, _ = chunk_items(b, c)
            slot_mid[max(0, s - 1)].append((0.0, dma))
            slot_mid[s].extend(comp[:4])
            slot_mid[min(s + 1, 15)].extend(comp[4:])

        place_chunk(0, 1, 0)
        place_chunk(0, 2, 2)
        place_chunk(0, 3, 4)
        for c in range(4):
            place_chunk(1, c, 6 + 2 * c)                          # s6, s8, s10, s12
        for jj in range(4):
            slot_late[2 * jj + 2].extend(outproj_items(0, jj))    # s2, s4, s6, s8
        slot_late[13].extend(outproj_items(1, 0))
        slot_late[14].extend(outproj_items(1, 1))
        slot_late[15].extend(outproj_items(1, 2))

        # chunk 0 of batch 0: head-pair 0's q/k plus all v go inline so
        # slot 0's attention (S and PV both) has its inputs; hp1's q/k
        # becomes slot-0 filler (deadline: slot 1).
        _, _, qk0, v0 = chunk_items(0, 0, split_x=True)
        qk0("k", 0)
        qk0("q", 0)
        for tt in range(4):
            v0(tt)
        c0_rest = [(QK_NS, lambda: qk0("k", 1)), (QK_NS, lambda: qk0("q", 1))]
        slot_mid[0] = c0_rest + slot_mid[0]

        carry = []
        for s in range(16):
            b, jj, hp = s // 8, (s % 8) // 2, s % 2
            lq = carry + slot_late[s]
            attn_slot(b, hp, jj, slot_mid[s], lq,
                      inline_op=(s == 15), credit0=(1500.0 if s == 0 else 400.0))
            carry = lq
        for cost, fn in carry:
            fn()

    nc.compile()
    return nc


def host_consts(t=T):
    pos = np.arange(t, dtype=np.float32)[:, None]           # [t, 1]
    j = np.arange(32, dtype=np.float32)[None, :]            # pair index
    theta = pos / np.power(np.float32(10000.0), 2.0 * j / np.float32(D))
    cos = np.cos(theta).astype(np.float32)                  # [t, 32]
    sin = np.sin(theta).astype(np.float32)
    # per-partition tables for the quadrant-interleaved layout (one head-pair
    # = 128 partitions; pattern repeats per 64-partition head)
    cos64 = np.zeros((64, t), np.float32)
    sin64 = np.zeros((64, t), np.float32)
    for quad in range(2):
        for slot in range(32):
            p = quad * 32 + slot
            jj = quad * 16 + (slot % 16)
            cos64[p] = cos[:, jj]
            sin64[p] = sin[:, jj] * (-1.0 if slot < 16 else 1.0)
    cos2 = np.tile(cos64, (2, 1)).astype(np.float16)
    sinS = np.tile(sin64, (2, 1)).astype(np.float16)
    r = np.arange(P)[:, None]
    cidx = np.arange(P)[None, :]
    ident = np.eye(P, dtype=np.float16)
    bias = np.where(r <= cidx, np.float16(0.0), np.float16(-400.0))
    mask0 = np.concatenate([ident, bias.astype(np.float16)], axis=1)
    return cos2, sinS, mask0


def make_in_maps(x, w_qkv, w_out):
    x = np.asarray(x, np.float32)
    w_qkv = np.asarray(w_qkv, np.float32)
    w_out = np.asarray(w_out, np.float32)
    cos2, sinS, mask0 = host_consts()
    perm = np.array(ROPE_PERM)
    in_maps = []
    xTs = []
    for bp in range(BP):
        xs = x[bp * B_LOC : (bp + 1) * B_LOC].reshape(B_LOC * T, C)
        xTs.append(np.ascontiguousarray(xs.T.astype(np.float16)))
    for c0 in range(NCORES):
        bp, hg = c0 // HG, c0 % HG
        qcols = np.concatenate(
            [hg * F + lh * D + perm for lh in range(HPC)])
        wq_p = w_qkv[:, 0:C][:, qcols].astype(np.float16)
        wk_p = w_qkv[:, C : 2 * C][:, qcols].astype(np.float16)
        wv_p = w_qkv[:, 2 * C :][:, hg * F : (hg + 1) * F].astype(np.float16)
        wo_p = w_out[hg * F : (hg + 1) * F, :].astype(np.float16)
        in_maps.append({
            "xT": xTs[bp],
            "wq": np.ascontiguousarray(wq_p),
            "wk": np.ascontiguousarray(wk_p),
            "wv": np.ascontiguousarray(wv_p),
            "wo": np.ascontiguousarray(wo_p),
            "cos2": cos2, "sinS": sinS, "mask0": mask0,
        })
    return in_maps


_REPL = {"cos2", "sinS", "mask0"}


class _Runner:
    """jit-once SPMD runner over jax.shard_map + the bass_exec custom call."""

    def __init__(self, nc, n_cores):
        import jax
        from jax.sharding import Mesh, PartitionSpec as PSpec
        from concourse import bass2jax

        bass2jax.install_neuronx_cc_hook()
        self.jax = jax
        self.n_cores = n_cores
        part_name = nc.partition_id_tensor.name if nc.partition_id_tensor else None
        in_names, out_names, out_avals, zero_outs = [], [], [], []
        for alloc in nc.m.functions[0].allocations:
            if not isinstance(alloc, mybir.MemoryLocationSet):
                continue
            name = alloc.memorylocations[0].name
            if alloc.kind == "ExternalInput":
                if name != part_name:
                    in_names.append(name)
            elif alloc.kind == "ExternalOutput":
                out_names.append(name)
                shape = tuple(alloc.tensor_shape)
                dtype = mybir.dt.np(alloc.dtype)
                out_avals.append(jax.core.ShapedArray(shape, dtype))
                zero_outs.append(np.zeros(shape, dtype))
        self.in_names, self.out_names = in_names, out_names
        self.out_avals, self.zero_outs = out_avals, zero_outs
        all_names = in_names + out_names + ([part_name] if part_name else [])

        def _body(*args):
            operands = list(args)
            if part_name is not None:
                operands.append(bass2jax.partition_id_tensor())
            outs = bass2jax._bass_exec_p.bind(
                *operands,
                out_avals=tuple(out_avals),
                in_names=tuple(all_names),
                out_names=tuple(out_names),
                lowering_input_output_aliases=(),
                sim_require_finite=False,
                sim_require_nnan=False,
                nc=nc,
            )
            return tuple(outs)

        try:
            from jax.experimental.shard_map import shard_map
        except ImportError:
            from jax.shard_map import shard_map
        devices = jax.devices()[:n_cores]
        self.mesh = Mesh(np.asarray(devices), ("core",))
        in_specs = tuple(
            PSpec() if nm in _REPL else PSpec("core") for nm in in_names
        ) + tuple(PSpec("core") for _ in out_names)
        out_specs = tuple(PSpec("core") for _ in out_names)
        self.fn = jax.jit(
            shard_map(_body, mesh=self.mesh, in_specs=in_specs,
                      out_specs=out_specs, check_rep=False),
            keep_unused=True,
        )

    def run(self, in_maps):
        args = []
        for nm in self.in_names:
            if nm in _REPL:
                args.append(np.asarray(in_maps[0][nm]))
            else:
                args.append(np.concatenate([np.asarray(m[nm]) for m in in_maps], axis=0))
        for z in self.zero_outs:
            args.append(np.zeros((self.n_cores * z.shape[0], *z.shape[1:]), z.dtype))
        outs = self.jax.block_until_ready(self.fn(*args))
        res = []
        for c in range(self.n_cores):
            res.append({
                nm: np.asarray(o).reshape(self.n_cores, *aval.shape)[c]
                for nm, aval, o in zip(self.out_names, self.out_avals, outs)
            })
        return res


_cache = {}


def kernel(x, w_qkv, w_out):
    if "runner" not in _cache:
        _cache["nc"] = build_nc()
        _cache["runner"] = _Runner(_cache["nc"], NCORES)
    in_maps = make_in_maps(x, w_qkv, w_out)
    results = _cache["runner"].run(in_maps)
    y = np.zeros((B, T, C), np.float32)
    for c0 in range(NCORES):
        bp = c0 // HG
        y[bp * B_LOC : (bp + 1) * B_LOC] += (
            results[c0]["y"].astype(np.float32).reshape(B_LOC, T, C)
        )
    return y


# revision 58
# speedup vs baseline: 1.0730x; 1.0730x over previous
import sys

sys.path.insert(0, "/opt/trn_rl_repo")

from contextlib import ExitStack

import numpy as np

import concourse.bass as bass
import concourse.tile as tile
from concourse import bacc
from concourse import mybir

B, T, C = 4, 2048, 1024
NH, D = 16, 64
NCORES = 8
BP = 2            # batch-pair shards (2 batches each)
HG = 4            # head-group shards (4 heads each)
B_LOC = 2         # batches per core
HPC = 4           # heads per core
HP = 2            # head-pairs per core
F = HPC * D       # per-core feature slice (256)
P = 128
TCH = 512         # token chunk (qkv) == q chunk (attention)
KT = 128          # k tile
f32 = mybir.dt.float32
f16 = mybir.dt.float16
AF = mybir.ActivationFunctionType

# quadrant-local rope layout: each 32-partition quadrant holds 16 x1 slots
# then 16 x2 slots; the rotation becomes a +/-16 shuffle within the quadrant.
ROPE_PERM = list(range(0, 16)) + list(range(32, 48)) + \
    list(range(16, 32)) + list(range(48, 64))
SHUF_MASK = [(i + 16) % 32 for i in range(32)]

MM_LABELS = []  # emission-order matmul labels (profiling aid)

# PE-time estimates (ns) for pacing filler work into attention slots
PEC = 1e9 / 2.4e9          # pe cycle at full speed
MM_OVH = 35.0
QK_NS = 8 * (512 * PEC + MM_OVH)
V_NS = 8 * (256 * PEC + MM_OVH)
OPH_NS = 2 * (512 * PEC + MM_OVH)
PACE = 1.35                # fillers pace at PE rate (PE-bound), not ACT rate


def build_nc():
    """One-core SPMD program: 2 batches x 4 heads (2 head-pairs).

    f16 compute throughout (fp8 cannot meet the accuracy gate on randn
    inputs: quantization error does not average out of attention). The
    attention loop is software-pipelined with S one k-tile ahead; the
    causal mask is folded into the score psum via an identity@bias
    matmul so no vector-engine mask op is needed; qkv + out-proj
    matmuls are paced into the slots by a PE-time credit model.
    """
    t = T
    bt = B_LOC * t            # 4096 tokens
    n_cc = C // P             # 8 contraction chunks

    nc = bacc.Bacc(None, target_bir_lowering=False)
    xT = nc.declare_dram_parameter("xT", [C, bt], f16, isOutput=False)
    wq = nc.declare_dram_parameter("wq", [C, F], f16, isOutput=False)
    wk = nc.declare_dram_parameter("wk", [C, F], f16, isOutput=False)
    wv = nc.declare_dram_parameter("wv", [C, F], f16, isOutput=False)
    wo = nc.declare_dram_parameter("wo", [F, C], f16, isOutput=False)
    cos2 = nc.declare_dram_parameter("cos2", [P, t], f16, isOutput=False)
    sinS = nc.declare_dram_parameter("sinS", [P, t], f16, isOutput=False)
    mask0 = nc.declare_dram_parameter("mask0", [P, 2 * P], f16, isOutput=False)
    y = nc.declare_dram_parameter("y", [bt, C], f16, isOutput=True)

    xT_t = xT.rearrange("(o p) n -> p o n", p=P)     # [128, 8, 4096]
    wq_r = wq.rearrange("(o p) f -> p o f", p=P)     # [128, 8, 256]
    wk_r = wk.rearrange("(o p) f -> p o f", p=P)
    wv_r = wv.rearrange("(o p) f -> p o f", p=P)
    wo_r = wo.rearrange("(hp p) c -> p hp c", p=P)   # [128, 2, 1024]

    MM_LABELS.clear()

    def MM(label, *a, **kw):
        MM_LABELS.append(label)
        nc.tensor.matmul(*a, **kw)

    with tile.TileContext(nc) as tc, ExitStack() as ctx:
        consts = ctx.enter_context(tc.tile_pool(name="consts", bufs=1))
        xpool = ctx.enter_context(tc.tile_pool(name="xt", bufs=3))
        stage = ctx.enter_context(tc.tile_pool(name="stage", bufs=3))
        ppool = ctx.enter_context(tc.tile_pool(name="pp", bufs=5))
        opool = ctx.enter_context(tc.tile_pool(name="op", bufs=2))
        ypool = ctx.enter_context(tc.tile_pool(name="yst", bufs=4))
        pss = ctx.enter_context(tc.tile_pool(name="pss", bufs=2, space="PSUM"))
        pso = ctx.enter_context(tc.tile_pool(name="pso", bufs=1, space="PSUM"))
        pmix = ctx.enter_context(tc.tile_pool(name="pmix", bufs=2, space="PSUM"))

        # constants; wk + chunk-0 x go out first (k items lead) so the PE
        # can start quickly; chunk 0 loads as ONE tile (fewer hwdge DMAs).
        wk_sb = consts.tile([P, n_cc, F], f16)
        nc.sync.dma_start(wk_sb[:, :, 0:P], wk_r[:, :, 0:P])
        xt0 = xpool.tile([P, n_cc, TCH], f16, tag="xt", name="xt0")
        nc.sync.dma_start(xt0[:, 0:4, :], xT_t[:, 0:4, 0:TCH])
        nc.sync.dma_start(xt0[:, 4:8, :], xT_t[:, 4:8, 0:TCH])
        nc.sync.dma_start(wk_sb[:, :, P:F], wk_r[:, :, P:F])
        wq_sb = consts.tile([P, n_cc, F], f16)
        nc.scalar.dma_start(wq_sb, wq_r)
        cos_sb = consts.tile([P, t], f16)
        nc.scalar.dma_start(cos_sb, cos2[:, :])
        sin_sb = consts.tile([P, t], f16)
        nc.scalar.dma_start(sin_sb, sinS[:, :])
        mask_sb = consts.tile([P, 2 * P], f16)
        nc.scalar.dma_start(mask_sb, mask0[:, :])
        wv_sb = consts.tile([P, n_cc, F], f16)
        nc.scalar.dma_start(wv_sb, wv_r)
        wo_sb = consts.tile([P, HP, C], f16)
        nc.scalar.dma_start(wo_sb, wo_r)

        # per-batch persistent tensors
        q_sb = [consts.tile([P, HP, t], f16, name=f"q{b}") for b in range(B_LOC)]
        k_sb = [consts.tile([P, HP, t], f16, name=f"k{b}") for b in range(B_LOC)]
        v1 = [consts.tile([P, t // KT, HPC, D + 1], f16, name=f"v1{b}")
              for b in range(B_LOC)]
        OT = [consts.tile([P, HP, t], f16, name=f"ot{b}") for b in range(B_LOC)]
        for b in range(B_LOC):
            nc.vector.memset(v1[b][:, :, :, D], 1.0)

        # ---- qkv filler items --------------------------------------------
        def rope_store(which, ft, b, c, ps):
            raw = stage.tile([P, TCH], f16, tag="raw")
            nc.vector.tensor_copy(raw, ps)
            rot = stage.tile([P, TCH], f16, tag="rot")
            nc.vector.stream_shuffle(rot, raw, SHUF_MASK)
            dst = (q_sb if which == "q" else k_sb)[b][:, ft, c * TCH : (c + 1) * TCH]
            cs = cos_sb[:, c * TCH : (c + 1) * TCH]
            sn = sin_sb[:, c * TCH : (c + 1) * TCH]
            nc.vector.tensor_mul(out=dst, in0=raw, in1=cs)
            tmp = stage.tile([P, TCH], f16, tag="tmp")
            nc.vector.tensor_mul(out=tmp, in0=rot, in1=sn)
            nc.vector.tensor_add(out=dst, in0=dst, in1=tmp)

        def chunk_items(b, c, split_x=False):
            """qkv for chunk c of batch b. Returns (dma_item, items);
            items are (pe_ns, fn) fillers; the DMA is a prefetch."""
            hold = {}
            items = []
            t0 = b * t + c * TCH
            dma_item = None

            if split_x:
                hold["xt"] = xt0           # preloaded (chunk 0 only)
            else:
                def dma_item():
                    xt = xpool.tile([P, n_cc, TCH], f16, tag="xt")
                    nc.scalar.dma_start(xt, xT_t[:, :, t0 : t0 + TCH])
                    hold["xt"] = xt

            def qk_item(which, ft):
                w_sb = wq_sb if which == "q" else wk_sb
                ps = pmix.tile([P, TCH], f32, tag="px")
                for cc in range(n_cc):
                    MM(
                        f"QKV:{b}.{c}:{which}{ft}",
                        ps, lhsT=w_sb[:, cc, ft * P : (ft + 1) * P],
                        rhs=hold["xt"][:, cc, :], start=(cc == 0),
                        stop=(cc == n_cc - 1),
                    )
                rope_store(which, ft, b, c, ps)

            def v_item(tt):
                ps = pmix.tile([P, TCH], f32, tag="px")
                for cc in range(n_cc):
                    MM(
                        f"V:{b}.{c}:{tt}",
                        ps[:, 0:F],
                        lhsT=hold["xt"][:, cc, tt * P : (tt + 1) * P],
                        rhs=wv_sb[:, cc, :], start=(cc == 0),
                        stop=(cc == n_cc - 1),
                    )
                src = ps[:, 0:F].rearrange("p (h d) -> p h d", d=D)
                nc.vector.tensor_copy(v1[b][:, c * 4 + tt, :, 0:D], src)

            for ft in range(HP):
                items.append((QK_NS, lambda ft=ft: qk_item("k", ft), None))
                items.append((QK_NS, lambda ft=ft: qk_item("q", ft), None))
            vitems = [(V_NS, (lambda tt=tt: v_item(tt)), c * 4 + tt)
                      for tt in range(TCH // P)]
            return dma_item, items, vitems, qk_item, v_item

        def op_item(b, jj, qt):
            # tail variant: copies alternate DVE/ACT so both engines drain
            # the final out-proj in parallel
            q0 = jj * TCH + qt * P
            r0 = b * t + q0
            ysb = ypool.tile([P, C], f16, tag="ysb")
            for nh in range(2):
                psY = pmix.tile([P, TCH], f32, tag="px")
                for hp in range(HP):
                    MM(
                        f"OP:{b}.{jj}:{qt}",
                        psY, lhsT=OT[b][:, hp, q0 : q0 + P],
                        rhs=wo_sb[:, hp, nh * 512 : (nh + 1) * 512],
                        start=(hp == 0), stop=(hp == HP - 1),
                    )
                nc.vector.tensor_copy(ysb[:, nh * 512 : (nh + 1) * 512], psY)
            nc.gpsimd.dma_start(y[r0 : r0 + P, :], ysb)

        def outproj_items(b, jj):
            units = []
            for qt in range(4):
                holder = {}

                def h0(qt=qt, holder=holder):
                    q0 = jj * TCH + qt * P
                    ysb = ypool.tile([P, C], f16, tag="ysb")
                    holder["ysb"] = ysb
                    psY = pmix.tile([P, TCH], f32, tag="px")
                    for hp in range(HP):
                        MM(f"OP:{b}.{jj}:{qt}", psY,
                           lhsT=OT[b][:, hp, q0 : q0 + P],
                           rhs=wo_sb[:, hp, 0:512],
                           start=(hp == 0), stop=(hp == HP - 1))
                    nc.vector.tensor_copy(ysb[:, 0:512], psY)

                def h1(qt=qt, holder=holder):
                    q0 = jj * TCH + qt * P
                    r0 = b * t + q0
                    ysb = holder["ysb"]
                    psY = pmix.tile([P, TCH], f32, tag="px")
                    for hp in range(HP):
                        MM(f"OP:{b}.{jj}:{qt}", psY,
                           lhsT=OT[b][:, hp, q0 : q0 + P],
                           rhs=wo_sb[:, hp, 512:1024],
                           start=(hp == 0), stop=(hp == HP - 1))
                    nc.vector.tensor_copy(ysb[:, 512:1024], psY)
                    nc.gpsimd.dma_start(y[r0 : r0 + P, :], ysb)

                units.append((OPH_NS, h0, None))
                units.append((OPH_NS, h1, None))
            return units

        # ---- attention slot ----------------------------------------------
        # software-pipelined: S one k-tile ahead of the exp stream; the
        # causal bias rides into the diag psum via identity@bias matmuls.
        def attn_slot(b, hp, jj, mid, late, inline_op=False, credit0=0.0):
            n_kt = 4 * jj + 4
            psO = pso.tile([P, 4, 256], f32, tag="po")
            psS_t = [None] * n_kt
            Pp_t = [None] * n_kt

            def S_emit(i):
                lo = max(0, (i - 4 * jj) * KT)
                diag = i >= 4 * jj
                psS = pss.tile([P, 2, TCH], f32, tag="ps")
                for h in range(2):
                    MM(
                        f"S:{b}.{hp}.{jj}:{i}",
                        psS[:, h, lo:],
                        lhsT=k_sb[b][h * D : (h + 1) * D, hp,
                                     i * KT : (i + 1) * KT],
                        rhs=q_sb[b][h * D : (h + 1) * D, hp,
                                    jj * TCH + lo : (jj + 1) * TCH],
                        start=True, stop=not diag,
                    )
                    if diag:
                        # psS[key, tok] += -400 above the causal diagonal:
                        # full-rank constant add (identity @ bias); exp then
                        # underflows to exact f16 zeros - no mask op needed
                        MM(
                            f"SM:{b}.{hp}.{jj}:{i}",
                            psS[:, h, lo : lo + P],
                            lhsT=mask_sb[:, 0:P], rhs=mask_sb[:, P : 2 * P],
                            start=False, stop=True,
                        )
                psS_t[i] = (psS, lo)
                return 2 * ((TCH - lo) * PEC + MM_OVH) + (176 if diag else 0)

            def exp_emit(i):
                psS, lo = psS_t[i]
                Pp = ppool.tile([P, 2, TCH], f16, tag="pp")
                Pp_t[i] = Pp
                nc.scalar.activation(Pp[:, :, lo:], psS[:, :, lo:],
                                     AF.Exp, scale=0.125)
                psS_t[i] = None
                return (2 * (TCH - lo) / 1.2 + 185.0) * PACE

            def PV_emit(i):
                Pp = Pp_t[i]
                lo = max(0, (i - 4 * jj) * KT)
                for qt in range(lo // P, 4):
                    for h in range(2):
                        MM(
                            f"PV:{b}.{hp}.{jj}:{i}",
                            psO[:, qt, h * 65 : h * 65 + 65],
                            lhsT=Pp[:, h, qt * P : (qt + 1) * P],
                            rhs=v1[b][:, i, hp * 2 + h, :],
                            start=(i == 0 and h == 0 and qt % 2 == 0),
                            stop=(i == 4 * jj + qt and h == 1 and qt % 2 == 1),
                        )
                return (4 - lo // P) * 2 * (65 * PEC + MM_OVH)

            def norm_pair(pr):
                rec = stage.tile([P, 2, 2], f32, tag=f"rec{pr}")
                nc.vector.reciprocal(
                    rec, psO[:, 2 * pr : 2 * pr + 2, D : 2 * D + 2 : D + 1])
                Osb = opool.tile([P, 2, 2, D], f16, tag=f"osb{pr}")
                src = psO[:, 2 * pr : 2 * pr + 2, 0 : 2 * (D + 1)].rearrange(
                    "p q (h e) -> p q h e", e=D + 1)[:, :, :, 0:D]
                nc.vector.tensor_mul(
                    out=Osb, in0=src,
                    in1=rec[:, :, :, None].to_broadcast((P, 2, 2, D)),
                )
                for qx in range(2):
                    qt = 2 * pr + qx
                    nc.sync.dma_start_transpose(
                        OT[b][:, hp, jj * TCH + qt * P : jj * TCH + (qt + 1) * P],
                        Osb[:, qx, :, :],
                    )
                    if inline_op:
                        op_item(b, jj, qt)

            credit = credit0
            mq, lq = list(mid), late
            late_gate = 1

            def drain(i):
                nonlocal credit
                while True:
                    if mq and credit >= mq[0][0]:
                        cost, fn, _ = mq.pop(0)
                    elif lq and i >= late_gate and credit >= lq[0][0]:
                        cost, fn, _ = lq.pop(0)
                    else:
                        break
                    fn()
                    credit -= cost
                # deadline pressure: mids must finish this slot — spread the
                # forced drain over the remaining iterations instead of
                # flushing everything after the last PV
                rem = n_kt - i - 1
                while len(mq) > 2 * rem:
                    cost, fn, _ = mq.pop(0)
                    fn()
                    credit -= cost

            def force_due(i):
                # v items tagged with a k-tile index must be emitted before
                # the PV matmul that reads that v tile
                nonlocal credit
                for it in [it for it in mq if it[2] is not None and it[2] <= i]:
                    mq.remove(it)
                    it[1]()
                    credit -= it[0]

            S_emit(0)
            for i in range(n_kt):
                if i + 1 < n_kt:
                    credit -= S_emit(i + 1)
                credit += exp_emit(i)
                drain(i)
                force_due(i)
                credit -= PV_emit(i)
                if i == 4 * jj + 1:
                    norm_pair(0)
            # norm first so the next slot's psO reuse isn't stuck behind
            # the flushed chunk work on the vector engine
            norm_pair(1)
            # chunk (mid) items have a hard deadline at the next slot: flush.
            # out-proj (late) leftovers carry forward to later slots.
            for cost, fn, _ in mq:
                fn()
                credit -= cost
            return max(credit, 0.0)

        # ---- schedule -----------------------------------------------------
        # slots s = b*8 + jj*2 + hp; chunk compute lands one slot before its
        # deadline with its x-load prefetched another slot earlier; out-proj
        # fills later slots (its OT transposes need time to land).
        slot_mid = {s: [] for s in range(16)}
        slot_late = {s: [] for s in range(16)}

        def place_chunk(b, c, s_qk, s_v):
            # q/k items two slots before their deadline (flush-safe); v
            # items ride in the deadline slot itself, force-emitted by
            # their k-tile due tags just ahead of the PV that reads them.
            dma, comp, vitems, _, _ = chunk_items(b, c)
            slot_mid[max(0, s_qk - 1)].append((0.0, dma, None))
            slot_mid[s_qk].extend(comp)
            slot_mid[s_v].extend(vitems)

        place_chunk(0, 1, 0, 2)
        place_chunk(0, 2, 2, 4)
        place_chunk(0, 3, 4, 6)
        for c in range(4):
            place_chunk(1, c, 6 + 2 * c, 8 + 2 * c)
        for jj in range(4):
            slot_late[2 * jj + 2].extend(outproj_items(0, jj))    # s2, s4, s6, s8
        slot_late[13].extend(outproj_items(1, 0))
        slot_late[14].extend(outproj_items(1, 1))
        slot_late[15].extend(outproj_items(1, 2))

        # chunk 0 of batch 0: head-pair 0's q/k go inline so attention can
        # start; v + hp1's q/k become slot-0 filler (v due-tagged).
        _, _, v0items, qk0, v0 = chunk_items(0, 0, split_x=True)
        qk0("k", 0)
        qk0("q", 0)
        c0_rest = [(QK_NS, (lambda: qk0("k", 1)), None),
                   (QK_NS, (lambda: qk0("q", 1)), None)]
        slot_mid[0] = v0items + c0_rest + slot_mid[0]

        carry = []
        for s in range(16):
            b, jj, hp = s // 8, (s % 8) // 2, s % 2
            lq = carry + slot_late[s]
            attn_slot(b, hp, jj, slot_mid[s], lq,
                      inline_op=(s == 15), credit0=(1500.0 if s == 0 else 400.0))
            carry = lq
        for cost, fn, _ in carry:
            fn()

    nc.compile()
    return nc


def host_consts(t=T):
    pos = np.arange(t, dtype=np.float32)[:, None]           # [t, 1]
    j = np.arange(32, dtype=np.float32)[None, :]            # pair index
    theta = pos / np.power(np.float32(10000.0), 2.0 * j / np.float32(D))
    cos = np.cos(theta).astype(np.float32)                  # [t, 32]
    sin = np.sin(theta).astype(np.float32)
    # per-partition tables for the quadrant-interleaved layout (one head-pair
    # = 128 partitions; pattern repeats per 64-partition head)
    cos64 = np.zeros((64, t), np.float32)
    sin64 = np.zeros((64, t), np.float32)
    for quad in range(2):
        for slot in range(32):
            p = quad * 32 + slot
            jj = quad * 16 + (slot % 16)
            cos64[p] = cos[:, jj]
            sin64[p] = sin[:, jj] * (-1.0 if slot < 16 else 1.0)
    cos2 = np.tile(cos64, (2, 1)).astype(np.float16)
    sinS = np.tile(sin64, (2, 1)).astype(np.float16)
    r = np.arange(P)[:, None]
    cidx = np.arange(P)[None, :]
    ident = np.eye(P, dtype=np.float16)
    bias = np.where(r <= cidx, np.float16(0.0), np.float16(-400.0))
    mask0 = np.concatenate([ident, bias.astype(np.float16)], axis=1)
    return cos2, sinS, mask0


def make_in_maps(x, w_qkv, w_out):
    x = np.asarray(x, np.float32)
    w_qkv = np.asarray(w_qkv, np.float32)
    w_out = np.asarray(w_out, np.float32)
    cos2, sinS, mask0 = host_consts()
    perm = np.array(ROPE_PERM)
    in_maps = []
    xTs = []
    for bp in range(BP):
        xs = x[bp * B_LOC : (bp + 1) * B_LOC].reshape(B_LOC * T, C)
        xTs.append(np.ascontiguousarray(xs.T.astype(np.float16)))
    for c0 in range(NCORES):
        bp, hg = c0 // HG, c0 % HG
        qcols = np.concatenate(
            [hg * F + lh * D + perm for lh in range(HPC)])
        wq_p = w_qkv[:, 0:C][:, qcols].astype(np.float16)
        wk_p = w_qkv[:, C : 2 * C][:, qcols].astype(np.float16)
        wv_p = w_qkv[:, 2 * C :][:, hg * F : (hg + 1) * F].astype(np.float16)
        wo_p = w_out[hg * F : (hg + 1) * F, :].astype(np.float16)
        in_maps.append({
            "xT": xTs[bp],
            "wq": np.ascontiguousarray(wq_p),
            "wk": np.ascontiguousarray(wk_p),
            "wv": np.ascontiguousarray(wv_p),
            "wo": np.ascontiguousarray(wo_p),
            "cos2": cos2, "sinS": sinS, "mask0": mask0,
        })
    return in_maps


_REPL = {"cos2", "sinS", "mask0"}


class _Runner:
    """jit-once SPMD runner over jax.shard_map + the bass_exec custom call."""

    def __init__(self, nc, n_cores):
        import jax
        from jax.sharding import Mesh, PartitionSpec as PSpec
        from concourse import bass2jax

        bass2jax.install_neuronx_cc_hook()
        self.jax = jax
        self.n_cores = n_cores
        part_name = nc.partition_id_tensor.name if nc.partition_id_tensor else None
        in_names, out_names, out_avals, zero_outs = [], [], [], []
        for alloc in nc.m.functions[0].allocations:
            if not isinstance(alloc, mybir.MemoryLocationSet):
                continue
            name = alloc.memorylocations[0].name
            if alloc.kind == "ExternalInput":
                if name != part_name:
                    in_names.append(name)
            elif alloc.kind == "ExternalOutput":
                out_names.append(name)
                shape = tuple(alloc.tensor_shape)
                dtype = mybir.dt.np(alloc.dtype)
                out_avals.append(jax.core.ShapedArray(shape, dtype))
                zero_outs.append(np.zeros(shape, dtype))
        self.in_names, self.out_names = in_names, out_names
        self.out_avals, self.zero_outs = out_avals, zero_outs
        all_names = in_names + out_names + ([part_name] if part_name else [])

        def _body(*args):
            operands = list(args)
            if part_name is not None:
                operands.append(bass2jax.partition_id_tensor())
            outs = bass2jax._bass_exec_p.bind(
                *operands,
                out_avals=tuple(out_avals),
                in_names=tuple(all_names),
                out_names=tuple(out_names),
                lowering_input_output_aliases=(),
                sim_require_finite=False,
                sim_require_nnan=False,
                nc=nc,
            )
            return tuple(outs)

        try:
            from jax.experimental.shard_map import shard_map
        except ImportError:
            from jax.shard_map import shard_map
        devices = jax.devices()[:n_cores]
        self.mesh = Mesh(np.asarray(devices), ("core",))
        in_specs = tuple(
            PSpec() if nm in _REPL else PSpec("core") for nm in in_names
        ) + tuple(PSpec("core") for _ in out_names)
        out_specs = tuple(PSpec("core") for _ in out_names)
        self.fn = jax.jit(
            shard_map(_body, mesh=self.mesh, in_specs=in_specs,
                      out_specs=out_specs, check_rep=False),
            keep_unused=True,
        )

    def run(self, in_maps):
        args = []
        for nm in self.in_names:
            if nm in _REPL:
                args.append(np.asarray(in_maps[0][nm]))
            else:
                args.append(np.concatenate([np.asarray(m[nm]) for m in in_maps], axis=0))
        for z in self.zero_outs:
            args.append(np.zeros((self.n_cores * z.shape[0], *z.shape[1:]), z.dtype))
        outs = self.jax.block_until_ready(self.fn(*args))
        res = []
        for c in range(self.n_cores):
            res.append({
                nm: np.asarray(o).reshape(self.n_cores, *aval.shape)[c]
                for nm, aval, o in zip(self.out_names, self.out_avals, outs)
            })
        return res


_cache = {}


def kernel(x, w_qkv, w_out):
    if "runner" not in _cache:
        _cache["nc"] = build_nc()
        _cache["runner"] = _Runner(_cache["nc"], NCORES)
    in_maps = make_in_maps(x, w_qkv, w_out)
    results = _cache["runner"].run(in_maps)
    y = np.zeros((B, T, C), np.float32)
    for c0 in range(NCORES):
        bp = c0 // HG
        y[bp * B_LOC : (bp + 1) * B_LOC] += (
            results[c0]["y"].astype(np.float32).reshape(B_LOC, T, C)
        )
    return y


# revision 66
# speedup vs baseline: 1.1085x; 1.0331x over previous
import sys

sys.path.insert(0, "/opt/trn_rl_repo")

from contextlib import ExitStack

import numpy as np

import concourse.bass as bass
import concourse.tile as tile
from concourse import bacc
from concourse import mybir

B, T, C = 4, 2048, 1024
NH, D = 16, 64
NCORES = 8
BP = 2            # batch-pair shards (2 batches each)
HG = 4            # head-group shards (4 heads each)
B_LOC = 2         # batches per core
HPC = 4           # heads per core
HP = 2            # head-pairs per core
F = HPC * D       # per-core feature slice (256)
P = 128
TCH = 512         # token chunk (qkv) == q chunk (attention)
KT = 128          # k tile
f32 = mybir.dt.float32
f16 = mybir.dt.float16
AF = mybir.ActivationFunctionType

# quadrant-local rope layout: each 32-partition quadrant holds 16 x1 slots
# then 16 x2 slots; the rotation becomes a +/-16 shuffle within the quadrant.
ROPE_PERM = list(range(0, 16)) + list(range(32, 48)) + \
    list(range(16, 32)) + list(range(48, 64))
SHUF_MASK = [(i + 16) % 32 for i in range(32)]

MM_LABELS = []  # emission-order matmul labels (profiling aid)

# PE-time estimates (ns) for pacing filler work into attention slots
PEC = 1e9 / 2.4e9          # pe cycle at full speed
MM_OVH = 35.0
QK_NS = 8 * (512 * PEC + MM_OVH)
V_NS = 8 * (256 * PEC + MM_OVH)
OPH_NS = 2 * (512 * PEC + MM_OVH)
PACE = 1.45                # fillers pace at PE rate (PE-bound), not ACT rate


def build_nc():
    """One-core SPMD program: 2 batches x 4 heads (2 head-pairs).

    f16 compute throughout (fp8 cannot meet the accuracy gate on randn
    inputs: quantization error does not average out of attention). The
    attention loop is software-pipelined with S one k-tile ahead; the
    causal mask is folded into the score psum via an identity@bias
    matmul so no vector-engine mask op is needed; qkv + out-proj
    matmuls are paced into the slots by a PE-time credit model.
    """
    t = T
    bt = B_LOC * t            # 4096 tokens
    n_cc = C // P             # 8 contraction chunks

    nc = bacc.Bacc(None, target_bir_lowering=False)
    xT = nc.declare_dram_parameter("xT", [C, bt], f16, isOutput=False)
    wq = nc.declare_dram_parameter("wq", [C, F], f16, isOutput=False)
    wk = nc.declare_dram_parameter("wk", [C, F], f16, isOutput=False)
    wv = nc.declare_dram_parameter("wv", [C, F], f16, isOutput=False)
    wo = nc.declare_dram_parameter("wo", [F, C], f16, isOutput=False)
    cos2 = nc.declare_dram_parameter("cos2", [P, t], f16, isOutput=False)
    sinS = nc.declare_dram_parameter("sinS", [P, t], f16, isOutput=False)
    mask0 = nc.declare_dram_parameter("mask0", [P, 2 * P], f16, isOutput=False)
    y = nc.declare_dram_parameter("y", [bt, C], f16, isOutput=True)

    xT_t = xT.rearrange("(o p) n -> p o n", p=P)     # [128, 8, 4096]
    wq_r = wq.rearrange("(o p) f -> p o f", p=P)     # [128, 8, 256]
    wk_r = wk.rearrange("(o p) f -> p o f", p=P)
    wv_r = wv.rearrange("(o p) f -> p o f", p=P)
    wo_r = wo.rearrange("(hp p) c -> p hp c", p=P)   # [128, 2, 1024]

    MM_LABELS.clear()

    def MM(label, *a, **kw):
        MM_LABELS.append(label)
        nc.tensor.matmul(*a, **kw)

    with tile.TileContext(nc) as tc, ExitStack() as ctx:
        consts = ctx.enter_context(tc.tile_pool(name="consts", bufs=1))
        xpool = ctx.enter_context(tc.tile_pool(name="xt", bufs=3))
        stage = ctx.enter_context(tc.tile_pool(name="stage", bufs=3))
        ppool = ctx.enter_context(tc.tile_pool(name="pp", bufs=5))
        opool = ctx.enter_context(tc.tile_pool(name="op", bufs=2))
        ypool = ctx.enter_context(tc.tile_pool(name="yst", bufs=4))
        pss = ctx.enter_context(tc.tile_pool(name="pss", bufs=2, space="PSUM"))
        pso = ctx.enter_context(tc.tile_pool(name="pso", bufs=1, space="PSUM"))
        pmix = ctx.enter_context(tc.tile_pool(name="pmix", bufs=2, space="PSUM"))

        # constants; wk + chunk-0 x go out first (k items lead) so the PE
        # can start quickly; chunk 0 loads as ONE tile (fewer hwdge DMAs).
        wk_sb = consts.tile([P, n_cc, F], f16)
        nc.sync.dma_start(wk_sb[:, :, 0:P], wk_r[:, :, 0:P])
        xt0 = xpool.tile([P, n_cc, TCH], f16, tag="xt", name="xt0")
        nc.sync.dma_start(xt0[:, 0:4, :], xT_t[:, 0:4, 0:TCH])
        nc.sync.dma_start(xt0[:, 4:8, :], xT_t[:, 4:8, 0:TCH])
        nc.sync.dma_start(wk_sb[:, :, P:F], wk_r[:, :, P:F])
        mask_sb = consts.tile([P, 2 * P], f16)
        nc.scalar.dma_start(mask_sb, mask0[:, :])
        wq_sb = consts.tile([P, n_cc, F], f16)
        nc.scalar.dma_start(wq_sb, wq_r)
        cos_sb = consts.tile([P, t], f16)
        nc.scalar.dma_start(cos_sb, cos2[:, :])
        sin_sb = consts.tile([P, t], f16)
        nc.scalar.dma_start(sin_sb, sinS[:, :])
        wv_sb = consts.tile([P, n_cc, F], f16)
        nc.scalar.dma_start(wv_sb, wv_r)
        wo_sb = consts.tile([P, HP, C], f16)
        nc.scalar.dma_start(wo_sb, wo_r)

        # per-batch persistent tensors
        q_sb = [consts.tile([P, HP, t], f16, name=f"q{b}") for b in range(B_LOC)]
        k_sb = [consts.tile([P, HP, t], f16, name=f"k{b}") for b in range(B_LOC)]
        v1 = [consts.tile([P, t // KT, HPC, D + 1], f16, name=f"v1{b}")
              for b in range(B_LOC)]
        OT = [consts.tile([P, HP, t], f16, name=f"ot{b}") for b in range(B_LOC)]
        for b in range(B_LOC):
            nc.vector.memset(v1[b][:, :, :, D], 1.0)

        # ---- qkv filler items --------------------------------------------
        def rope_store(which, ft, b, c, ps):
            raw = stage.tile([P, TCH], f16, tag="raw")
            nc.vector.tensor_copy(raw, ps)
            rot = stage.tile([P, TCH], f16, tag="rot")
            nc.vector.stream_shuffle(rot, raw, SHUF_MASK)
            dst = (q_sb if which == "q" else k_sb)[b][:, ft, c * TCH : (c + 1) * TCH]
            cs = cos_sb[:, c * TCH : (c + 1) * TCH]
            sn = sin_sb[:, c * TCH : (c + 1) * TCH]
            nc.vector.tensor_mul(out=dst, in0=raw, in1=cs)
            tmp = stage.tile([P, TCH], f16, tag="tmp")
            nc.vector.tensor_mul(out=tmp, in0=rot, in1=sn)
            nc.vector.tensor_add(out=dst, in0=dst, in1=tmp)

        def chunk_items(b, c, split_x=False):
            """qkv for chunk c of batch b. Returns (dma_item, items);
            items are (pe_ns, fn) fillers; the DMA is a prefetch."""
            hold = {}
            items = []
            t0 = b * t + c * TCH
            dma_item = None

            if split_x:
                hold["xt"] = xt0           # preloaded (chunk 0 only)
            else:
                def dma_item():
                    xt = xpool.tile([P, n_cc, TCH], f16, tag="xt")
                    nc.scalar.dma_start(xt, xT_t[:, :, t0 : t0 + TCH])
                    hold["xt"] = xt

            def qk_item(which, ft):
                w_sb = wq_sb if which == "q" else wk_sb
                ps = pmix.tile([P, TCH], f32, tag="px")
                for cc in range(n_cc):
                    MM(
                        f"QKV:{b}.{c}:{which}{ft}",
                        ps, lhsT=w_sb[:, cc, ft * P : (ft + 1) * P],
                        rhs=hold["xt"][:, cc, :], start=(cc == 0),
                        stop=(cc == n_cc - 1),
                    )
                rope_store(which, ft, b, c, ps)

            def v_item(tt):
                ps = pmix.tile([P, TCH], f32, tag="px")
                for cc in range(n_cc):
                    MM(
                        f"V:{b}.{c}:{tt}",
                        ps[:, 0:F],
                        lhsT=hold["xt"][:, cc, tt * P : (tt + 1) * P],
                        rhs=wv_sb[:, cc, :], start=(cc == 0),
                        stop=(cc == n_cc - 1),
                    )
                src = ps[:, 0:F].rearrange("p (h d) -> p h d", d=D)
                nc.vector.tensor_copy(v1[b][:, c * 4 + tt, :, 0:D], src)

            for ft in range(HP):
                items.append((QK_NS, lambda ft=ft: qk_item("k", ft), None))
                items.append((QK_NS, lambda ft=ft: qk_item("q", ft), None))
            vitems = [(V_NS, (lambda tt=tt: v_item(tt)), c * 4 + tt)
                      for tt in range(TCH // P)]
            return dma_item, items, vitems, qk_item, v_item

        def op_item(b, jj, qt):
            # tail variant: copies alternate DVE/ACT so both engines drain
            # the final out-proj in parallel
            q0 = jj * TCH + qt * P
            r0 = b * t + q0
            ysb = ypool.tile([P, C], f16, tag="ysb")
            for nh in range(2):
                psY = pmix.tile([P, TCH], f32, tag="px")
                for hp in range(HP):
                    MM(
                        f"OP:{b}.{jj}:{qt}",
                        psY, lhsT=OT[b][:, hp, q0 : q0 + P],
                        rhs=wo_sb[:, hp, nh * 512 : (nh + 1) * 512],
                        start=(hp == 0), stop=(hp == HP - 1),
                    )
                nc.vector.tensor_copy(ysb[:, nh * 512 : (nh + 1) * 512], psY)
            nc.gpsimd.dma_start(y[r0 : r0 + P, :], ysb)

        def outproj_items(b, jj):
            units = []
            for qt in range(4):
                holder = {}

                def h0(qt=qt, holder=holder):
                    q0 = jj * TCH + qt * P
                    ysb = ypool.tile([P, C], f16, tag="ysb")
                    holder["ysb"] = ysb
                    psY = pmix.tile([P, TCH], f32, tag="px")
                    for hp in range(HP):
                        MM(f"OP:{b}.{jj}:{qt}", psY,
                           lhsT=OT[b][:, hp, q0 : q0 + P],
                           rhs=wo_sb[:, hp, 0:512],
                           start=(hp == 0), stop=(hp == HP - 1))
                    nc.vector.tensor_copy(ysb[:, 0:512], psY)

                def h1(qt=qt, holder=holder):
                    q0 = jj * TCH + qt * P
                    r0 = b * t + q0
                    ysb = holder["ysb"]
                    psY = pmix.tile([P, TCH], f32, tag="px")
                    for hp in range(HP):
                        MM(f"OP:{b}.{jj}:{qt}", psY,
                           lhsT=OT[b][:, hp, q0 : q0 + P],
                           rhs=wo_sb[:, hp, 512:1024],
                           start=(hp == 0), stop=(hp == HP - 1))
                    nc.vector.tensor_copy(ysb[:, 512:1024], psY)
                    nc.gpsimd.dma_start(y[r0 : r0 + P, :], ysb)

                units.append((OPH_NS, h0, None))
                units.append((OPH_NS, h1, None))
            return units

        # ---- attention slot ----------------------------------------------
        # software-pipelined: S one k-tile ahead of the exp stream; the
        # causal bias rides into the diag psum via identity@bias matmuls.
        def attn_slot(b, hp, jj, mid, late, inline_op=False, credit0=0.0):
            n_kt = 4 * jj + 4
            psO = pso.tile([P, 4, 256], f32, tag="po")
            psS_t = [None] * n_kt
            Pp_t = [None] * n_kt

            def S_emit(i):
                lo = max(0, (i - 4 * jj) * KT)
                diag = i >= 4 * jj
                psS = pss.tile([P, 2, TCH], f32, tag="ps")
                for h in range(2):
                    MM(
                        f"S:{b}.{hp}.{jj}:{i}",
                        psS[:, h, lo:],
                        lhsT=k_sb[b][h * D : (h + 1) * D, hp,
                                     i * KT : (i + 1) * KT],
                        rhs=q_sb[b][h * D : (h + 1) * D, hp,
                                    jj * TCH + lo : (jj + 1) * TCH],
                        start=True, stop=not diag,
                    )
                    if diag:
                        # psS[key, tok] += -400 above the causal diagonal:
                        # full-rank constant add (identity @ bias); exp then
                        # underflows to exact f16 zeros - no mask op needed
                        MM(
                            f"SM:{b}.{hp}.{jj}:{i}",
                            psS[:, h, lo : lo + P],
                            lhsT=mask_sb[:, 0:P], rhs=mask_sb[:, P : 2 * P],
                            start=False, stop=True,
                        )
                psS_t[i] = (psS, lo)
                return 2 * ((TCH - lo) * PEC + MM_OVH) + (176 if diag else 0)

            def exp_emit(i):
                psS, lo = psS_t[i]
                Pp = ppool.tile([P, 2, TCH], f16, tag="pp")
                Pp_t[i] = Pp
                nc.scalar.activation(Pp[:, :, lo:], psS[:, :, lo:],
                                     AF.Exp, scale=0.125)
                psS_t[i] = None
                return (2 * (TCH - lo) / 1.2 + 185.0) * PACE

            def PV_emit(i):
                Pp = Pp_t[i]
                lo = max(0, (i - 4 * jj) * KT)
                for qt in range(lo // P, 4):
                    for h in range(2):
                        MM(
                            f"PV:{b}.{hp}.{jj}:{i}",
                            psO[:, qt, h * 65 : h * 65 + 65],
                            lhsT=Pp[:, h, qt * P : (qt + 1) * P],
                            rhs=v1[b][:, i, hp * 2 + h, :],
                            start=(i == 0 and h == 0 and qt % 2 == 0),
                            stop=(i == 4 * jj + qt and h == 1 and qt % 2 == 1),
                        )
                return (4 - lo // P) * 2 * (65 * PEC + MM_OVH)

            def norm_pair(pr):
                rec = stage.tile([P, 2, 2], f32, tag=f"rec{pr}")
                nc.vector.reciprocal(
                    rec, psO[:, 2 * pr : 2 * pr + 2, D : 2 * D + 2 : D + 1])
                Osb = opool.tile([P, 2, 2, D], f16, tag=f"osb{pr}")
                src = psO[:, 2 * pr : 2 * pr + 2, 0 : 2 * (D + 1)].rearrange(
                    "p q (h e) -> p q h e", e=D + 1)[:, :, :, 0:D]
                nc.vector.tensor_mul(
                    out=Osb, in0=src,
                    in1=rec[:, :, :, None].to_broadcast((P, 2, 2, D)),
                )
                for qx in range(2):
                    qt = 2 * pr + qx
                    nc.sync.dma_start_transpose(
                        OT[b][:, hp, jj * TCH + qt * P : jj * TCH + (qt + 1) * P],
                        Osb[:, qx, :, :],
                    )
                if inline_op:
                    for qx in range(2):
                        op_item(b, jj, 2 * pr + qx)

            credit = credit0
            mq, lq = list(mid), late
            late_gate = 1

            def drain(i):
                nonlocal credit
                while True:
                    if mq and credit >= mq[0][0]:
                        cost, fn, _ = mq.pop(0)
                    elif lq and i >= late_gate and credit >= lq[0][0]:
                        cost, fn, _ = lq.pop(0)
                    else:
                        break
                    fn()
                    credit -= cost
                # deadline pressure: mids must finish this slot — spread the
                # forced drain over the remaining iterations instead of
                # flushing everything after the last PV
                rem = n_kt - i - 1
                while len(mq) > 2 * rem:
                    cost, fn, _ = mq.pop(0)
                    fn()
                    credit -= cost

            def force_due(i):
                # v items tagged with a k-tile index must be emitted before
                # the PV matmul that reads that v tile
                nonlocal credit
                for it in [it for it in mq if it[2] is not None and it[2] <= i]:
                    mq.remove(it)
                    it[1]()
                    credit -= it[0]

            S_emit(0)
            for i in range(n_kt):
                if i + 1 < n_kt:
                    credit -= S_emit(i + 1)
                credit += exp_emit(i)
                drain(i)
                force_due(i)
                credit -= PV_emit(i)
                if i == 4 * jj + 1:
                    norm_pair(0)
            # norm first so the next slot's psO reuse isn't stuck behind
            # the flushed chunk work on the vector engine
            norm_pair(1)
            # chunk (mid) items have a hard deadline at the next slot: flush.
            # out-proj (late) leftovers carry forward to later slots.
            for cost, fn, _ in mq:
                fn()
                credit -= cost
            return max(credit, 0.0)

        # ---- schedule -----------------------------------------------------
        # slots s = b*8 + jj*2 + hp; chunk compute lands one slot before its
        # deadline with its x-load prefetched another slot earlier; out-proj
        # fills later slots (its OT transposes need time to land).
        slot_mid = {s: [] for s in range(16)}
        slot_late = {s: [] for s in range(16)}

        def place_chunk(b, c, s_qk, s_v):
            # q/k items two slots before their deadline (flush-safe); v
            # items ride in the deadline slot itself, force-emitted by
            # their k-tile due tags just ahead of the PV that reads them.
            dma, comp, vitems, _, _ = chunk_items(b, c)
            slot_mid[max(0, s_qk - 1)].append((0.0, dma, None))
            slot_mid[s_qk].extend(comp)
            slot_mid[s_v].extend(vitems)

        place_chunk(0, 1, 0, 2)
        place_chunk(0, 2, 2, 4)
        place_chunk(0, 3, 4, 6)
        for c in range(4):
            place_chunk(1, c, 7 + 2 * c, 8 + 2 * c)
        for jj in range(4):
            slot_late[2 * jj + 3].extend(outproj_items(0, jj))    # s3, s5, s7, s9
        slot_late[13].extend(outproj_items(1, 0))
        slot_late[14].extend(outproj_items(1, 1))
        slot_late[15].extend(outproj_items(1, 2))

        # chunk 0 of batch 0: head-pair 0's q/k go inline so attention can
        # start; v + hp1's q/k become slot-0 filler (v due-tagged).
        _, _, v0items, qk0, v0 = chunk_items(0, 0, split_x=True)
        qk0("k", 0)
        qk0("q", 0)
        c0_rest = [(QK_NS, (lambda: qk0("k", 1)), None),
                   (QK_NS, (lambda: qk0("q", 1)), None)]
        slot_mid[0] = v0items + c0_rest + slot_mid[0]

        carry = []
        for s in range(16):
            b, jj, hp = s // 8, (s % 8) // 2, s % 2
            lq = carry + slot_late[s]
            attn_slot(b, hp, jj, slot_mid[s], lq,
                      inline_op=(s == 15), credit0=(1500.0 if s == 0 else 400.0))
            carry = lq
        for cost, fn, _ in carry:
            fn()

    nc.compile()
    return nc


def host_consts(t=T):
    pos = np.arange(t, dtype=np.float32)[:, None]           # [t, 1]
    j = np.arange(32, dtype=np.float32)[None, :]            # pair index
    theta = pos / np.power(np.float32(10000.0), 2.0 * j / np.float32(D))
    cos = np.cos(theta).astype(np.float32)                  # [t, 32]
    sin = np.sin(theta).astype(np.float32)
    # per-partition tables for the quadrant-interleaved layout (one head-pair
    # = 128 partitions; pattern repeats per 64-partition head)
    cos64 = np.zeros((64, t), np.float32)
    sin64 = np.zeros((64, t), np.float32)
    for quad in range(2):
        for slot in range(32):
            p = quad * 32 + slot
            jj = quad * 16 + (slot % 16)
            cos64[p] = cos[:, jj]
            sin64[p] = sin[:, jj] * (-1.0 if slot < 16 else 1.0)
    cos2 = np.tile(cos64, (2, 1)).astype(np.float16)
    sinS = np.tile(sin64, (2, 1)).astype(np.float16)
    r = np.arange(P)[:, None]
    cidx = np.arange(P)[None, :]
    ident = np.eye(P, dtype=np.float16)
    bias = np.where(r <= cidx, np.float16(0.0), np.float16(-400.0))
    mask0 = np.concatenate([ident, bias.astype(np.float16)], axis=1)
    return cos2, sinS, mask0


def make_in_maps(x, w_qkv, w_out):
    x = np.asarray(x, np.float32)
    w_qkv = np.asarray(w_qkv, np.float32)
    w_out = np.asarray(w_out, np.float32)
    cos2, sinS, mask0 = host_consts()
    perm = np.array(ROPE_PERM)
    in_maps = []
    xTs = []
    for bp in range(BP):
        xs = x[bp * B_LOC : (bp + 1) * B_LOC].reshape(B_LOC * T, C)
        xTs.append(np.ascontiguousarray(xs.T.astype(np.float16)))
    for c0 in range(NCORES):
        bp, hg = c0 // HG, c0 % HG
        qcols = np.concatenate(
            [hg * F + lh * D + perm for lh in range(HPC)])
        wq_p = w_qkv[:, 0:C][:, qcols].astype(np.float16)
        wk_p = w_qkv[:, C : 2 * C][:, qcols].astype(np.float16)
        wv_p = w_qkv[:, 2 * C :][:, hg * F : (hg + 1) * F].astype(np.float16)
        wo_p = w_out[hg * F : (hg + 1) * F, :].astype(np.float16)
        in_maps.append({
            "xT": xTs[bp],
            "wq": np.ascontiguousarray(wq_p),
            "wk": np.ascontiguousarray(wk_p),
            "wv": np.ascontiguousarray(wv_p),
            "wo": np.ascontiguousarray(wo_p),
            "cos2": cos2, "sinS": sinS, "mask0": mask0,
        })
    return in_maps


_REPL = {"cos2", "sinS", "mask0"}


class _Runner:
    """jit-once SPMD runner over jax.shard_map + the bass_exec custom call."""

    def __init__(self, nc, n_cores):
        import jax
        from jax.sharding import Mesh, PartitionSpec as PSpec
        from concourse import bass2jax

        bass2jax.install_neuronx_cc_hook()
        self.jax = jax
        self.n_cores = n_cores
        part_name = nc.partition_id_tensor.name if nc.partition_id_tensor else None
        in_names, out_names, out_avals, zero_outs = [], [], [], []
        for alloc in nc.m.functions[0].allocations:
            if not isinstance(alloc, mybir.MemoryLocationSet):
                continue
            name = alloc.memorylocations[0].name
            if alloc.kind == "ExternalInput":
                if name != part_name:
                    in_names.append(name)
            elif alloc.kind == "ExternalOutput":
                out_names.append(name)
                shape = tuple(alloc.tensor_shape)
                dtype = mybir.dt.np(alloc.dtype)
                out_avals.append(jax.core.ShapedArray(shape, dtype))
                zero_outs.append(np.zeros(shape, dtype))
        self.in_names, self.out_names = in_names, out_names
        self.out_avals, self.zero_outs = out_avals, zero_outs
        all_names = in_names + out_names + ([part_name] if part_name else [])

        def _body(*args):
            operands = list(args)
            if part_name is not None:
                operands.append(bass2jax.partition_id_tensor())
            outs = bass2jax._bass_exec_p.bind(
                *operands,
                out_avals=tuple(out_avals),
                in_names=tuple(all_names),
                out_names=tuple(out_names),
                lowering_input_output_aliases=(),
                sim_require_finite=False,
                sim_require_nnan=False,
                nc=nc,
            )
            return tuple(outs)

        try:
            from jax.experimental.shard_map import shard_map
        except ImportError:
            from jax.shard_map import shard_map
        devices = jax.devices()[:n_cores]
        self.mesh = Mesh(np.asarray(devices), ("core",))
        in_specs = tuple(
            PSpec() if nm in _REPL else PSpec("core") for nm in in_names
        ) + tuple(PSpec("core") for _ in out_names)
        out_specs = tuple(PSpec("core") for _ in out_names)
        self.fn = jax.jit(
            shard_map(_body, mesh=self.mesh, in_specs=in_specs,
                      out_specs=out_specs, check_rep=False),
            keep_unused=True,
        )

    def run(self, in_maps):
        args = []
        for nm in self.in_names:
            if nm in _REPL:
                args.append(np.asarray(in_maps[0][nm]))
            else:
                args.append(np.concatenate([np.asarray(m[nm]) for m in in_maps], axis=0))
        for z in self.zero_outs:
            args.append(np.zeros((self.n_cores * z.shape[0], *z.shape[1:]), z.dtype))
        outs = self.jax.block_until_ready(self.fn(*args))
        res = []
        for c in range(self.n_cores):
            res.append({
                nm: np.asarray(o).reshape(self.n_cores, *aval.shape)[c]
                for nm, aval, o in zip(self.out_names, self.out_avals, outs)
            })
        return res


_cache = {}


def kernel(x, w_qkv, w_out):
    if "runner" not in _cache:
        _cache["nc"] = build_nc()
        _cache["runner"] = _Runner(_cache["nc"], NCORES)
    in_maps = make_in_maps(x, w_qkv, w_out)
    results = _cache["runner"].run(in_maps)
    y = np.zeros((B, T, C), np.float32)
    for c0 in range(NCORES):
        bp = c0 // HG
        y[bp * B_LOC : (bp + 1) * B_LOC] += (
            results[c0]["y"].astype(np.float32).reshape(B_LOC, T, C)
        )
    return y


# revision 69
# speedup vs baseline: 1.1118x; 1.0029x over previous
import sys

sys.path.insert(0, "/opt/trn_rl_repo")

from contextlib import ExitStack

import numpy as np

import concourse.bass as bass
import concourse.tile as tile
from concourse import bacc
from concourse import mybir

B, T, C = 4, 2048, 1024
NH, D = 16, 64
NCORES = 8
BP = 2            # batch-pair shards (2 batches each)
HG = 4            # head-group shards (4 heads each)
B_LOC = 2         # batches per core
HPC = 4           # heads per core
HP = 2            # head-pairs per core
F = HPC * D       # per-core feature slice (256)
P = 128
TCH = 512         # token chunk (qkv) == q chunk (attention)
KT = 128          # k tile
f32 = mybir.dt.float32
f16 = mybir.dt.float16
AF = mybir.ActivationFunctionType

# quadrant-local rope layout: each 32-partition quadrant holds 16 x1 slots
# then 16 x2 slots; the rotation becomes a +/-16 shuffle within the quadrant.
ROPE_PERM = list(range(0, 16)) + list(range(32, 48)) + \
    list(range(16, 32)) + list(range(48, 64))
SHUF_MASK = [(i + 16) % 32 for i in range(32)]

MM_LABELS = []  # emission-order matmul labels (profiling aid)

# PE-time estimates (ns) for pacing filler work into attention slots
PEC = 1e9 / 2.4e9          # pe cycle at full speed
MM_OVH = 35.0
QK_NS = 8 * (512 * PEC + MM_OVH)
V_NS = 8 * (256 * PEC + MM_OVH)
OPH_NS = 2 * (512 * PEC + MM_OVH)
PACE = 1.45                # fillers pace at PE rate (PE-bound), not ACT rate


def build_nc():
    """One-core SPMD program: 2 batches x 4 heads (2 head-pairs).

    f16 compute throughout (fp8 cannot meet the accuracy gate on randn
    inputs: quantization error does not average out of attention). The
    attention loop is software-pipelined with S one k-tile ahead; the
    causal mask is folded into the score psum via an identity@bias
    matmul so no vector-engine mask op is needed; qkv + out-proj
    matmuls are paced into the slots by a PE-time credit model.
    """
    t = T
    bt = B_LOC * t            # 4096 tokens
    n_cc = C // P             # 8 contraction chunks

    nc = bacc.Bacc(None, target_bir_lowering=False)
    xT = nc.declare_dram_parameter("xT", [C, bt], f16, isOutput=False)
    wq = nc.declare_dram_parameter("wq", [C, F], f16, isOutput=False)
    wk = nc.declare_dram_parameter("wk", [C, F], f16, isOutput=False)
    wv = nc.declare_dram_parameter("wv", [C, F], f16, isOutput=False)
    wo = nc.declare_dram_parameter("wo", [F, C], f16, isOutput=False)
    cos2 = nc.declare_dram_parameter("cos2", [P, t], f16, isOutput=False)
    sinS = nc.declare_dram_parameter("sinS", [P, t], f16, isOutput=False)
    mask0 = nc.declare_dram_parameter("mask0", [P, 2 * P], f16, isOutput=False)
    y = nc.declare_dram_parameter("y", [bt, C], f16, isOutput=True)

    xT_t = xT.rearrange("(o p) n -> p o n", p=P)     # [128, 8, 4096]
    wq_r = wq.rearrange("(o p) f -> p o f", p=P)     # [128, 8, 256]
    wk_r = wk.rearrange("(o p) f -> p o f", p=P)
    wv_r = wv.rearrange("(o p) f -> p o f", p=P)
    wo_r = wo.rearrange("(hp p) c -> p hp c", p=P)   # [128, 2, 1024]

    MM_LABELS.clear()

    def MM(label, *a, **kw):
        MM_LABELS.append(label)
        nc.tensor.matmul(*a, **kw)

    with tile.TileContext(nc) as tc, ExitStack() as ctx:
        consts = ctx.enter_context(tc.tile_pool(name="consts", bufs=1))
        xpool = ctx.enter_context(tc.tile_pool(name="xt", bufs=3))
        stage = ctx.enter_context(tc.tile_pool(name="stage", bufs=3))
        ppool = ctx.enter_context(tc.tile_pool(name="pp", bufs=5))
        opool = ctx.enter_context(tc.tile_pool(name="op", bufs=2))
        ypool = ctx.enter_context(tc.tile_pool(name="yst", bufs=4))
        pss = ctx.enter_context(tc.tile_pool(name="pss", bufs=2, space="PSUM"))
        pso = ctx.enter_context(tc.tile_pool(name="pso", bufs=1, space="PSUM"))
        pmix = ctx.enter_context(tc.tile_pool(name="pmix", bufs=2, space="PSUM"))

        # constants; wk + chunk-0 x go out first (k items lead) so the PE
        # can start quickly; chunk 0 loads as ONE tile (fewer hwdge DMAs).
        wk_sb = consts.tile([P, n_cc, F], f16)
        nc.sync.dma_start(wk_sb[:, :, 0:P], wk_r[:, :, 0:P])
        xt0 = xpool.tile([P, n_cc, TCH], f16, tag="xt", name="xt0")
        nc.sync.dma_start(xt0[:, 0:4, :], xT_t[:, 0:4, 0:TCH])
        nc.sync.dma_start(xt0[:, 4:8, :], xT_t[:, 4:8, 0:TCH])
        nc.sync.dma_start(wk_sb[:, :, P:F], wk_r[:, :, P:F])
        mask_sb = consts.tile([P, 2 * P], f16)
        nc.scalar.dma_start(mask_sb, mask0[:, :])
        wq_sb = consts.tile([P, n_cc, F], f16)
        nc.scalar.dma_start(wq_sb, wq_r)
        cos_sb = consts.tile([P, t], f16)
        nc.scalar.dma_start(cos_sb, cos2[:, :])
        sin_sb = consts.tile([P, t], f16)
        nc.scalar.dma_start(sin_sb, sinS[:, :])
        wv_sb = consts.tile([P, n_cc, F], f16)
        nc.scalar.dma_start(wv_sb, wv_r)
        wo_sb = consts.tile([P, HP, C], f16)
        nc.scalar.dma_start(wo_sb, wo_r)

        # per-batch persistent tensors
        q_sb = [consts.tile([P, HP, t], f16, name=f"q{b}") for b in range(B_LOC)]
        k_sb = [consts.tile([P, HP, t], f16, name=f"k{b}") for b in range(B_LOC)]
        v1 = [consts.tile([P, t // KT, HPC, D + 1], f16, name=f"v1{b}")
              for b in range(B_LOC)]
        OT = [consts.tile([P, HP, t], f16, name=f"ot{b}") for b in range(B_LOC)]
        for b in range(B_LOC):
            nc.vector.memset(v1[b][:, :, :, D], 1.0)

        # ---- qkv filler items --------------------------------------------
        def rope_store(which, ft, b, c, ps):
            raw = stage.tile([P, TCH], f16, tag="raw")
            nc.vector.tensor_copy(raw, ps)
            rot = stage.tile([P, TCH], f16, tag="rot")
            nc.vector.stream_shuffle(rot, raw, SHUF_MASK)
            dst = (q_sb if which == "q" else k_sb)[b][:, ft, c * TCH : (c + 1) * TCH]
            cs = cos_sb[:, c * TCH : (c + 1) * TCH]
            sn = sin_sb[:, c * TCH : (c + 1) * TCH]
            nc.vector.tensor_mul(out=dst, in0=raw, in1=cs)
            tmp = stage.tile([P, TCH], f16, tag="tmp")
            nc.vector.tensor_mul(out=tmp, in0=rot, in1=sn)
            nc.vector.tensor_add(out=dst, in0=dst, in1=tmp)

        def chunk_items(b, c, split_x=False):
            """qkv for chunk c of batch b. Returns (dma_item, items);
            items are (pe_ns, fn) fillers; the DMA is a prefetch."""
            hold = {}
            items = []
            t0 = b * t + c * TCH
            dma_item = None

            if split_x:
                hold["xt"] = xt0           # preloaded (chunk 0 only)
            else:
                def dma_item():
                    xt = xpool.tile([P, n_cc, TCH], f16, tag="xt")
                    nc.scalar.dma_start(xt, xT_t[:, :, t0 : t0 + TCH])
                    hold["xt"] = xt

            def qk_item(which, ft):
                w_sb = wq_sb if which == "q" else wk_sb
                ps = pmix.tile([P, TCH], f32, tag="px")
                for cc in range(n_cc):
                    MM(
                        f"QKV:{b}.{c}:{which}{ft}",
                        ps, lhsT=w_sb[:, cc, ft * P : (ft + 1) * P],
                        rhs=hold["xt"][:, cc, :], start=(cc == 0),
                        stop=(cc == n_cc - 1),
                    )
                rope_store(which, ft, b, c, ps)

            def v_item(tt):
                ps = pmix.tile([P, TCH], f32, tag="px")
                for cc in range(n_cc):
                    MM(
                        f"V:{b}.{c}:{tt}",
                        ps[:, 0:F],
                        lhsT=hold["xt"][:, cc, tt * P : (tt + 1) * P],
                        rhs=wv_sb[:, cc, :], start=(cc == 0),
                        stop=(cc == n_cc - 1),
                    )
                src = ps[:, 0:F].rearrange("p (h d) -> p h d", d=D)
                nc.vector.tensor_copy(v1[b][:, c * 4 + tt, :, 0:D], src)

            for ft in range(HP):
                items.append((QK_NS, lambda ft=ft: qk_item("k", ft), None))
                items.append((QK_NS, lambda ft=ft: qk_item("q", ft), None))
            vitems = [(V_NS, (lambda tt=tt: v_item(tt)), c * 4 + tt)
                      for tt in range(TCH // P)]
            return dma_item, items, vitems, qk_item, v_item

        def op_item(b, jj, qt):
            # tail variant: copies alternate DVE/ACT so both engines drain
            # the final out-proj in parallel
            q0 = jj * TCH + qt * P
            r0 = b * t + q0
            ysb = ypool.tile([P, C], f16, tag="ysb")
            for nh in range(2):
                psY = pmix.tile([P, TCH], f32, tag="px")
                for hp in range(HP):
                    MM(
                        f"OP:{b}.{jj}:{qt}",
                        psY, lhsT=OT[b][:, hp, q0 : q0 + P],
                        rhs=wo_sb[:, hp, nh * 512 : (nh + 1) * 512],
                        start=(hp == 0), stop=(hp == HP - 1),
                    )
                nc.vector.tensor_copy(ysb[:, nh * 512 : (nh + 1) * 512], psY)
            # tail path: SP queue skips the ~1us swdge descriptor generation
            nc.sync.dma_start(y[r0 : r0 + P, :], ysb)

        def outproj_items(b, jj):
            units = []
            for qt in range(4):
                holder = {}

                def h0(qt=qt, holder=holder):
                    q0 = jj * TCH + qt * P
                    ysb = ypool.tile([P, C], f16, tag="ysb")
                    holder["ysb"] = ysb
                    psY = pmix.tile([P, TCH], f32, tag="px")
                    for hp in range(HP):
                        MM(f"OP:{b}.{jj}:{qt}", psY,
                           lhsT=OT[b][:, hp, q0 : q0 + P],
                           rhs=wo_sb[:, hp, 0:512],
                           start=(hp == 0), stop=(hp == HP - 1))
                    nc.vector.tensor_copy(ysb[:, 0:512], psY)

                def h1(qt=qt, holder=holder):
                    q0 = jj * TCH + qt * P
                    r0 = b * t + q0
                    ysb = holder["ysb"]
                    psY = pmix.tile([P, TCH], f32, tag="px")
                    for hp in range(HP):
                        MM(f"OP:{b}.{jj}:{qt}", psY,
                           lhsT=OT[b][:, hp, q0 : q0 + P],
                           rhs=wo_sb[:, hp, 512:1024],
                           start=(hp == 0), stop=(hp == HP - 1))
                    nc.vector.tensor_copy(ysb[:, 512:1024], psY)
                    nc.gpsimd.dma_start(y[r0 : r0 + P, :], ysb)

                units.append((OPH_NS, h0, None))
                units.append((OPH_NS, h1, None))
            return units

        # ---- attention slot ----------------------------------------------
        # software-pipelined: S one k-tile ahead of the exp stream; the
        # causal bias rides into the diag psum via identity@bias matmuls.
        def attn_slot(b, hp, jj, mid, late, inline_op=False, credit0=0.0):
            n_kt = 4 * jj + 4
            psO = pso.tile([P, 4, 256], f32, tag="po")
            psS_t = [None] * n_kt
            Pp_t = [None] * n_kt

            def S_emit(i):
                lo = max(0, (i - 4 * jj) * KT)
                diag = i >= 4 * jj
                psS = pss.tile([P, 2, TCH], f32, tag="ps")
                for h in range(2):
                    MM(
                        f"S:{b}.{hp}.{jj}:{i}",
                        psS[:, h, lo:],
                        lhsT=k_sb[b][h * D : (h + 1) * D, hp,
                                     i * KT : (i + 1) * KT],
                        rhs=q_sb[b][h * D : (h + 1) * D, hp,
                                    jj * TCH + lo : (jj + 1) * TCH],
                        start=True, stop=not diag,
                    )
                    if diag:
                        # psS[key, tok] += -400 above the causal diagonal:
                        # full-rank constant add (identity @ bias); exp then
                        # underflows to exact f16 zeros - no mask op needed
                        MM(
                            f"SM:{b}.{hp}.{jj}:{i}",
                            psS[:, h, lo : lo + P],
                            lhsT=mask_sb[:, 0:P], rhs=mask_sb[:, P : 2 * P],
                            start=False, stop=True,
                        )
                psS_t[i] = (psS, lo)
                return 2 * ((TCH - lo) * PEC + MM_OVH) + (176 if diag else 0)

            def exp_emit(i):
                psS, lo = psS_t[i]
                Pp = ppool.tile([P, 2, TCH], f16, tag="pp")
                Pp_t[i] = Pp
                nc.scalar.activation(Pp[:, :, lo:], psS[:, :, lo:],
                                     AF.Exp, scale=0.125)
                psS_t[i] = None
                return (2 * (TCH - lo) / 1.2 + 185.0) * PACE

            def PV_emit(i):
                Pp = Pp_t[i]
                lo = max(0, (i - 4 * jj) * KT)
                for qt in range(lo // P, 4):
                    for h in range(2):
                        MM(
                            f"PV:{b}.{hp}.{jj}:{i}",
                            psO[:, qt, h * 65 : h * 65 + 65],
                            lhsT=Pp[:, h, qt * P : (qt + 1) * P],
                            rhs=v1[b][:, i, hp * 2 + h, :],
                            start=(i == 0 and h == 0 and qt % 2 == 0),
                            stop=(i == 4 * jj + qt and h == 1 and qt % 2 == 1),
                        )
                return (4 - lo // P) * 2 * (65 * PEC + MM_OVH)

            def norm_pair(pr):
                rec = stage.tile([P, 2, 2], f32, tag=f"rec{pr}")
                nc.vector.reciprocal(
                    rec, psO[:, 2 * pr : 2 * pr + 2, D : 2 * D + 2 : D + 1])
                Osb = opool.tile([P, 2, 2, D], f16, tag=f"osb{pr}")
                src = psO[:, 2 * pr : 2 * pr + 2, 0 : 2 * (D + 1)].rearrange(
                    "p q (h e) -> p q h e", e=D + 1)[:, :, :, 0:D]
                nc.vector.tensor_mul(
                    out=Osb, in0=src,
                    in1=rec[:, :, :, None].to_broadcast((P, 2, 2, D)),
                )
                for qx in range(2):
                    qt = 2 * pr + qx
                    nc.sync.dma_start_transpose(
                        OT[b][:, hp, jj * TCH + qt * P : jj * TCH + (qt + 1) * P],
                        Osb[:, qx, :, :],
                    )
                if inline_op:
                    for qx in range(2):
                        op_item(b, jj, 2 * pr + qx)

            credit = credit0
            mq, lq = list(mid), late
            late_gate = 0

            def drain(i):
                nonlocal credit
                while True:
                    if mq and credit >= mq[0][0]:
                        cost, fn, _ = mq.pop(0)
                    elif lq and i >= late_gate and credit >= lq[0][0]:
                        cost, fn, _ = lq.pop(0)
                    else:
                        break
                    fn()
                    credit -= cost
                # deadline pressure: mids must finish this slot — spread the
                # forced drain over the remaining iterations instead of
                # flushing everything after the last PV
                rem = n_kt - i - 1
                while len(mq) > 2 * rem:
                    cost, fn, _ = mq.pop(0)
                    fn()
                    credit -= cost

            def force_due(i):
                # v items tagged with a k-tile index must be emitted before
                # the PV matmul that reads that v tile
                nonlocal credit
                for it in [it for it in mq if it[2] is not None and it[2] <= i]:
                    mq.remove(it)
                    it[1]()
                    credit -= it[0]

            S_emit(0)
            for i in range(n_kt):
                if i + 1 < n_kt:
                    credit -= S_emit(i + 1)
                credit += exp_emit(i)
                drain(i)
                force_due(i)
                credit -= PV_emit(i)
                if i == 4 * jj + 1:
                    norm_pair(0)
            # norm first so the next slot's psO reuse isn't stuck behind
            # the flushed chunk work on the vector engine
            norm_pair(1)
            # chunk (mid) items have a hard deadline at the next slot: flush.
            # out-proj (late) leftovers carry forward to later slots.
            for cost, fn, _ in mq:
                fn()
                credit -= cost
            return max(credit, 0.0)

        # ---- schedule -----------------------------------------------------
        # slots s = b*8 + jj*2 + hp; chunk compute lands one slot before its
        # deadline with its x-load prefetched another slot earlier; out-proj
        # fills later slots (its OT transposes need time to land).
        slot_mid = {s: [] for s in range(16)}
        slot_late = {s: [] for s in range(16)}

        def place_chunk(b, c, s_qk, s_v):
            # q/k items two slots before their deadline (flush-safe); v
            # items ride in the deadline slot itself, force-emitted by
            # their k-tile due tags just ahead of the PV that reads them.
            dma, comp, vitems, _, _ = chunk_items(b, c)
            slot_mid[max(0, s_qk - 1)].append((0.0, dma, None))
            slot_mid[s_qk].extend(comp)
            slot_mid[s_v].extend(vitems)

        place_chunk(0, 1, 0, 2)
        place_chunk(0, 2, 2, 4)
        place_chunk(0, 3, 4, 6)
        for c in range(4):
            place_chunk(1, c, 7 + 2 * c, 8 + 2 * c)
        for jj in range(4):
            slot_late[2 * jj + 3].extend(outproj_items(0, jj))    # s3, s5, s7, s9
        slot_late[13].extend(outproj_items(1, 0))
        slot_late[14].extend(outproj_items(1, 1))
        slot_late[15].extend(outproj_items(1, 2))

        # chunk 0 of batch 0: head-pair 0's q/k go inline so attention can
        # start; v + hp1's q/k become slot-0 filler (v due-tagged).
        _, _, v0items, qk0, v0 = chunk_items(0, 0, split_x=True)
        qk0("k", 0)
        qk0("q", 0)
        c0_rest = [(QK_NS, (lambda: qk0("k", 1)), None),
                   (QK_NS, (lambda: qk0("q", 1)), None)]
        slot_mid[0] = v0items + c0_rest + slot_mid[0]

        carry = []
        for s in range(16):
            b, jj, hp = s // 8, (s % 8) // 2, s % 2
            lq = carry + slot_late[s]
            attn_slot(b, hp, jj, slot_mid[s], lq,
                      inline_op=(s == 15), credit0=(1500.0 if s == 0 else 400.0))
            carry = lq
        for cost, fn, _ in carry:
            fn()

    nc.compile()
    return nc


def host_consts(t=T):
    pos = np.arange(t, dtype=np.float32)[:, None]           # [t, 1]
    j = np.arange(32, dtype=np.float32)[None, :]            # pair index
    theta = pos / np.power(np.float32(10000.0), 2.0 * j / np.float32(D))
    cos = np.cos(theta).astype(np.float32)                  # [t, 32]
    sin = np.sin(theta).astype(np.float32)
    # per-partition tables for the quadrant-interleaved layout (one head-pair
    # = 128 partitions; pattern repeats per 64-partition head)
    cos64 = np.zeros((64, t), np.float32)
    sin64 = np.zeros((64, t), np.float32)
    for quad in range(2):
        for slot in range(32):
            p = quad * 32 + slot
            jj = quad * 16 + (slot % 16)
            cos64[p] = cos[:, jj]
            sin64[p] = sin[:, jj] * (-1.0 if slot < 16 else 1.0)
    cos2 = np.tile(cos64, (2, 1)).astype(np.float16)
    sinS = np.tile(sin64, (2, 1)).astype(np.float16)
    r = np.arange(P)[:, None]
    cidx = np.arange(P)[None, :]
    ident = np.eye(P, dtype=np.float16)
    bias = np.where(r <= cidx, np.float16(0.0), np.float16(-400.0))
    mask0 = np.concatenate([ident, bias.astype(np.float16)], axis=1)
    return cos2, sinS, mask0


def make_in_maps(x, w_qkv, w_out):
    x = np.asarray(x, np.float32)
    w_qkv = np.asarray(w_qkv, np.float32)
    w_out = np.asarray(w_out, np.float32)
    cos2, sinS, mask0 = host_consts()
    perm = np.array(ROPE_PERM)
    in_maps = []
    xTs = []
    for bp in range(BP):
        xs = x[bp * B_LOC : (bp + 1) * B_LOC].reshape(B_LOC * T, C)
        xTs.append(np.ascontiguousarray(xs.T.astype(np.float16)))
    for c0 in range(NCORES):
        bp, hg = c0 // HG, c0 % HG
        qcols = np.concatenate(
            [hg * F + lh * D + perm for lh in range(HPC)])
        wq_p = w_qkv[:, 0:C][:, qcols].astype(np.float16)
        wk_p = w_qkv[:, C : 2 * C][:, qcols].astype(np.float16)
        wv_p = w_qkv[:, 2 * C :][:, hg * F : (hg + 1) * F].astype(np.float16)
        wo_p = w_out[hg * F : (hg + 1) * F, :].astype(np.float16)
        in_maps.append({
            "xT": xTs[bp],
            "wq": np.ascontiguousarray(wq_p),
            "wk": np.ascontiguousarray(wk_p),
            "wv": np.ascontiguousarray(wv_p),
            "wo": np.ascontiguousarray(wo_p),
            "cos2": cos2, "sinS": sinS, "mask0": mask0,
        })
    return in_maps


_REPL = {"cos2", "sinS", "mask0"}


class _Runner:
    """jit-once SPMD runner over jax.shard_map + the bass_exec custom call."""

    def __init__(self, nc, n_cores):
        import jax
        from jax.sharding import Mesh, PartitionSpec as PSpec
        from concourse import bass2jax

        bass2jax.install_neuronx_cc_hook()
        self.jax = jax
        self.n_cores = n_cores
        part_name = nc.partition_id_tensor.name if nc.partition_id_tensor else None
        in_names, out_names, out_avals, zero_outs = [], [], [], []
        for alloc in nc.m.functions[0].allocations:
            if not isinstance(alloc, mybir.MemoryLocationSet):
                continue
            name = alloc.memorylocations[0].name
            if alloc.kind == "ExternalInput":
                if name != part_name:
                    in_names.append(name)
            elif alloc.kind == "ExternalOutput":
                out_names.append(name)
                shape = tuple(alloc.tensor_shape)
                dtype = mybir.dt.np(alloc.dtype)
                out_avals.append(jax.core.ShapedArray(shape, dtype))
                zero_outs.append(np.zeros(shape, dtype))
        self.in_names, self.out_names = in_names, out_names
        self.out_avals, self.zero_outs = out_avals, zero_outs
        all_names = in_names + out_names + ([part_name] if part_name else [])

        def _body(*args):
            operands = list(args)
            if part_name is not None:
                operands.append(bass2jax.partition_id_tensor())
            outs = bass2jax._bass_exec_p.bind(
                *operands,
                out_avals=tuple(out_avals),
                in_names=tuple(all_names),
                out_names=tuple(out_names),
                lowering_input_output_aliases=(),
                sim_require_finite=False,
                sim_require_nnan=False,
                nc=nc,
            )
            return tuple(outs)

        try:
            from jax.experimental.shard_map import shard_map
        except ImportError:
            from jax.shard_map import shard_map
        devices = jax.devices()[:n_cores]
        self.mesh = Mesh(np.asarray(devices), ("core",))
        in_specs = tuple(
            PSpec() if nm in _REPL else PSpec("core") for nm in in_names
        ) + tuple(PSpec("core") for _ in out_names)
        out_specs = tuple(PSpec("core") for _ in out_names)
        self.fn = jax.jit(
            shard_map(_body, mesh=self.mesh, in_specs=in_specs,
                      out_specs=out_specs, check_rep=False),
            keep_unused=True,
        )

    def run(self, in_maps):
        args = []
        for nm in self.in_names:
            if nm in _REPL:
                args.append(np.asarray(in_maps[0][nm]))
            else:
                args.append(np.concatenate([np.asarray(m[nm]) for m in in_maps], axis=0))
        for z in self.zero_outs:
            args.append(np.zeros((self.n_cores * z.shape[0], *z.shape[1:]), z.dtype))
        outs = self.jax.block_until_ready(self.fn(*args))
        res = []
        for c in range(self.n_cores):
            res.append({
                nm: np.asarray(o).reshape(self.n_cores, *aval.shape)[c]
                for nm, aval, o in zip(self.out_names, self.out_avals, outs)
            })
        return res


_cache = {}


def kernel(x, w_qkv, w_out):
    if "runner" not in _cache:
        _cache["nc"] = build_nc()
        _cache["runner"] = _Runner(_cache["nc"], NCORES)
    in_maps = make_in_maps(x, w_qkv, w_out)
    results = _cache["runner"].run(in_maps)
    y = np.zeros((B, T, C), np.float32)
    for c0 in range(NCORES):
        bp = c0 // HG
        y[bp * B_LOC : (bp + 1) * B_LOC] += (
            results[c0]["y"].astype(np.float32).reshape(B_LOC, T, C)
        )
    return y
